# revision 1
# baseline (speedup 1.0000x reference)
"""Trainium2 Bass kernel for nn_CNN_Casual (LeNet-ish CNN, B=8192).

Pure data parallel over 8 NeuronCores: 1024 samples per core, parameters
replicated, one SPMD Bass program. Per core, samples are processed in
blocks of 128 (the TensorEngine stationary-operand width):

  conv1  : the host gathers x into overlapping windows (8 input rows x 16
           cols = K 128) and folds sigmoid(mask) into a per-window Toeplitz
           weight matrix [128, 480] (exact - the mask is elementwise on the
           input and conv is linear). Per (row-window, col-half): one fp16
           matmul, stationary = data [128, 128 samples], moving = weights
           [128, 480 = 4 output rows x 10 ch x 12 cols].
  pool1  : fused 2x2 max of the PSUM tile. Split between a DVE-direct
           6D-AP reduce_max (XY over the pair dims) and an ACT psum->fp16
           copy + two DVE fp16 tensor_max stages (2x_1P mode), chosen per
           tile to balance the two engines (GPSIMD cannot run TT/reduce
           through walrus, and cannot read PSUM).
  T1     : PE transposes (fp16, 1 cyc/row) into a shared [120, 512] PSUM
           tile; one relu(x + b1) eviction per 4 transposes (ScalarE
           activation or DVE scalar_tensor_tensor, alternating). The
           per-channel conv bias commutes with max-pool so it is applied
           here, where it is per-PARTITION (free on the eviction op).
  conv2  : Toeplitz master [120, 7*160] = [Z,W4,W3,W2,W1,W0,Z] in fp16;
           output-row-pair group g accumulates 6 uniform-width (N=320)
           matmuls in PSUM; zero blocks keep every matmul wide enough to
           hide the weight load and make has_written semantics uniform.
  pool2/T2: same pattern -> f_all [80, 1024] (fp16) per 256-sample pair.
  fc1    : weights stationary [80, 50] x 4 groups, moving = f slices
           [80, 2x128]; relu+bias -> fc1o [50, 256] fp16.
  fc2    : data stationary [50, 128], moving weights [50, 10].
  softmax: per block, DVE computes t1 = (logits - rowmax) + fc2_b (any
           per-sample shift is exact for log_softmax); a half-core batched
           epilogue does one Exp, one windowed reduce_sum, one Ln and the
           final subtracts, so the ScalarE activation table loads once.

dtypes: conv inputs/weights and pooled activations are fp16 (|x| <= ~30,
11-bit mantissa keeps the end-to-end max relative error ~4e-4 vs the fp32
reference); PSUM accumulation is always fp32; pooling/softmax arithmetic
is fp32 except where noted. DMA: one input DMA per 256 samples (512B
runs), weights ~1.9MB once, one output DMA per 512 samples.
"""

from contextlib import ExitStack

import numpy as np

import concourse.mybir as mybir
import concourse.tile as tile
from concourse import bacc
from concourse.bass_utils import run_bass_kernel_spmd

F32 = mybir.dt.float32
FP16 = mybir.dt.float16
AF = mybir.ActivationFunctionType
AX = mybir.AxisListType

N_CORES = 8
B_TOTAL = 8192
B_CORE = B_TOTAL // N_CORES  # 1024


# --------------------------------------------------------------------------
# Host-side weight preparation (tiny tensors; exact rearrangement only)
# --------------------------------------------------------------------------
def _prep_weights(mask_w, conv1_w, conv1_b, conv2_w, conv2_b, fc1_w, fc1_b,
                  fc2_w, fc2_b):
    f32 = np.float32
    sig = (1.0 / (1.0 + np.exp(-mask_w.astype(f32)))).astype(f32)  # [28,28]

    # conv1 Toeplitz windows with mask folded in.
    # window (w,h): input rows 4w..4w+7, cols 12h..12h+15 (K = 8*16 = 128)
    # col index of the moving matrix: dp*120 + o*12 + ql
    #   (output row p = 4w+dp, output col q = 12h+ql)
    w1b = np.zeros((128, 480), f32)
    oo = np.arange(10)
    for dp in range(4):
        for ki in range(5):
            i = dp + ki
            for kj in range(5):
                for ql in range(12):
                    j = ql + kj
                    w1b[i * 16 + j, dp * 120 + oo * 12 + ql] = \
                        conv1_w[:, 0, ki, kj]
    w1m = np.empty((12, 128, 480), np.float16)
    for w in range(6):
        for h in range(2):
            win = sig[4 * w:4 * w + 8, 12 * h:12 * h + 16].reshape(128, 1)
            w1m[w * 2 + h] = (w1b * win).astype(np.float16)
    w1m = np.ascontiguousarray(w1m.transpose(1, 0, 2).reshape(128, 5760))

    # conv2 master Toeplitz: blocks [Z, W4, W3, W2, W1, W0, Z], each [120,160]
    # row index (c, j) = c*12 + j; col index (o2, q2) = o2*8 + q2
    w2m = np.zeros((120, 7, 160), np.float16)
    o2 = np.arange(20)
    for k in range(5):
        blk = 5 - k
        for c in range(10):
            for kj in range(5):
                for q2 in range(8):
                    j = q2 + kj
                    w2m[c * 12 + j, blk, o2 * 8 + q2] = conv2_w[:, c, k, kj]
    w2m_flat = np.ascontiguousarray(w2m.reshape(120, 7 * 160))

    # fc1 weights per pooled-row group p': rows (o2, s2), torch flatten order
    # of the conv2 activations is (o2, p', s2).
    fc1w4 = fc1_w.reshape(50, 20, 4, 4)  # [m, o2, p', s2]
    wfc1 = np.concatenate(
        [np.ascontiguousarray(fc1w4[:, :, p, :].reshape(50, 80).T)
         for p in range(4)],
        axis=1,
    )  # [80, 200]

    # const blob 1 (fp32): ident | bc2 | b1 | b2 | bf1  -> [128, 141]
    cst = np.zeros((128, 141), f32)
    cst[:, 0:128] = np.eye(128, dtype=f32)
    # constant stabilizing shift for log_softmax (exact: any per-sample
    # constant cancels); logits stay well inside fp32 exp range
    cst[:, 128:138] = np.tile(fc2_b.astype(f32).reshape(1, 10) - 10.0,
                              (128, 1))
    cst[0:120, 138] = np.repeat(conv1_b.astype(f32), 12)
    cst[0:80, 139] = np.repeat(conv2_b.astype(f32), 4)
    cst[0:50, 140] = fc1_b.astype(f32)

    # const blob 2 (fp16): fc2_w.T | wfc1 -> [80, 210]
    wfcb = np.zeros((80, 210), np.float16)
    wfcb[0:50, 0:10] = fc2_w.T.astype(np.float16)
    wfcb[:, 10:210] = wfc1.astype(np.float16)

    idb = np.eye(128).astype(np.float16)
    return dict(w1m=w1m, w2m=w2m_flat, wfcb=wfcb, cst=cst, idb=idb)


# --------------------------------------------------------------------------
# Device program
# --------------------------------------------------------------------------
def _build(b_core):
    assert b_core % 256 == 0
    n_pair = b_core // 256

    nc = bacc.Bacc("TRN2", target_bir_lowering=False, debug=False,
                   num_devices=N_CORES)

    xw_d = nc.dram_tensor("xw", [12, 128, b_core], FP16,
                          kind="ExternalInput").ap()
    w1m_d = nc.dram_tensor("w1m", [128, 5760], FP16,
                           kind="ExternalInput").ap()
    w2m_d = nc.dram_tensor("w2m", [120, 1120], FP16, kind="ExternalInput").ap()
    wfcb_d = nc.dram_tensor("wfcb", [80, 210], FP16, kind="ExternalInput").ap()
    cst_d = nc.dram_tensor("cst", [128, 141], F32, kind="ExternalInput").ap()
    idb_d = nc.dram_tensor("idb", [128, 128], FP16, kind="ExternalInput").ap()
    y = nc.dram_tensor("y", [b_core, 10], F32, kind="ExternalOutput").ap()

    with tile.TileContext(nc) as tc, ExitStack() as ctx:
        consts = ctx.enter_context(tc.tile_pool(name="consts", bufs=1))
        identb = consts.tile([128, 128], FP16)
        nc.sync.dma_start(identb[:], idb_d)
        w1m_sb = consts.tile([128, 5760], FP16)
        w2m_sb = consts.tile([120, 1120], FP16)
        wfcb_sb = consts.tile([80, 210], FP16)
        cst_sb = consts.tile([128, 141], F32)

        ident = cst_sb[:, 0:128]
        bc2_sb = cst_sb[:, 128:138]
        b1_sb = cst_sb[0:120, 138:139]
        b2_sb = cst_sb[0:80, 139:140]
        bf1_sb = cst_sb[0:50, 140:141]
        wfc2_sb = wfcb_sb[0:50, 0:10]
        wfc1_sb = wfcb_sb[:, 10:210]

        zeros = consts.tile([120, 512], FP16)
        nc.vector.memset(zeros[:], 0.0)

        xw_pool = ctx.enter_context(tc.tile_pool(name="xw", bufs=3))
        ps1_pool = ctx.enter_context(tc.tile_pool(name="ps1", bufs=3,
                                                  space="PSUM"))
        tmp_pool = ctx.enter_context(tc.tile_pool(name="tmpb", bufs=6))
        prp_pool = ctx.enter_context(tc.tile_pool(name="prp", bufs=4))
        tpw_pool = ctx.enter_context(tc.tile_pool(name="tpw", bufs=2,
                                                  space="PSUM"))
        x2_pool = ctx.enter_context(tc.tile_pool(name="x2", bufs=2))
        ps2_pool = ctx.enter_context(tc.tile_pool(name="ps2", bufs=2,
                                                  space="PSUM"))
        psf_pool = ctx.enter_context(tc.tile_pool(name="psf", bufs=1,
                                                  space="PSUM"))
        f_pool = ctx.enter_context(tc.tile_pool(name="fp", bufs=2))
        fc1o_pool = ctx.enter_context(tc.tile_pool(name="fc1o", bufs=2))
        sm_pool = ctx.enter_context(tc.tile_pool(name="sm", bufs=3))
        t1_all = consts.tile([128, 10 * 2 * n_pair], F32)

        SUB, ADD, MAX = (mybir.AluOpType.subtract, mybir.AluOpType.add,
                         mybir.AluOpType.max)

        def relu_bias_evict(idx, dst, src_ps, bias, width):
            """dst = relu(src_ps + bias) rounded to f32r; alternate engines."""
            if idx % 2 == 0:
                nc.scalar.activation(dst, src_ps, AF.Relu, bias=bias)
            else:
                nc.vector.scalar_tensor_tensor(
                    dst, src_ps, bias, zeros[:dst.shape[0], :width],
                    op0=ADD, op1=MAX)

        for pair in range(n_pair):
            f_all = f_pool.tile([80, 1024], FP16, name="f_all", tag="f_all")
            fview = f_all.rearrange("p (h g n) -> p g h n", h=2, g=4, n=128)
            xwcat = xw_pool.tile([128, 3072], FP16, name="xwcat", tag="xw")
            deng = nc.sync if pair % 2 == 0 else nc.scalar
            deng.dma_start(
                xwcat.rearrange("p (t n) -> p t n", t=12),
                xw_d[:, :, pair * 256:pair * 256 + 256]
                .rearrange("t p n -> p t n"))
            for half in range(2):
                blk = pair * 2 + half
                b0 = blk * 128
                prp_t = []
                for w in range(6):
                    prp = prp_pool.tile([128, 240], FP16, name="prp_t",
                                        tag="prp")
                    prp_t.append(prp)
                    for h in range(2):
                        t = w * 2 + h
                        if pair == 0 and half == 0:
                            eng = nc.sync if t % 2 == 0 else nc.scalar
                            eng.dma_start(
                                w1m_sb[:, t * 480:(t + 1) * 480],
                                w1m_d[:, t * 480:(t + 1) * 480])
                        ps1 = ps1_pool.tile([128, 480], F32, name="ps1_t",
                                            tag="ps1")
                        nc.tensor.matmul(ps1[:],
                                         xwcat[:, t * 256 + half * 128:
                                               t * 256 + half * 128 + 128],
                                         w1m_sb[:, t * 480:(t + 1) * 480],
                                         start=True, stop=True)
                        # pool 2x2: reduce over (tr, tc) of
                        # [p, u, o, m, tr, tc]; dst strided into prp
                        dst = prp.rearrange("p (u o q) -> p u o q",
                                            u=2, o=10)[:, :, :, 6 * h:6 * h + 6]
                        if t in (0, 3, 6, 9):
                            src = ps1.rearrange(
                                "p (u tr o m tc) -> p u o m tr tc",
                                u=2, tr=2, o=10, m=6)
                            nc.vector.reduce_max(dst, src, axis=AX.XY)
                        else:
                            tmp = tmp_pool.tile([128, 480], FP16,
                                                name="tmpb_t", tag="tmpb")
                            nc.scalar.copy(tmp[:], ps1[:])
                            tv = tmp.rearrange("p (u tr c) -> p u tr c",
                                               u=2, tr=2)
                            rm = tmp_pool.tile([128, 240], FP16,
                                               name="rm_t", tag="rm")
                            rmv = rm.rearrange("p (u c) -> p u c", u=2)
                            nc.vector.tensor_max(rmv, tv[:, :, 0],
                                                 tv[:, :, 1])
                            rv = rm.rearrange("p (u o m tc) -> p u o m tc",
                                              u=2, o=10, m=6)
                            nc.vector.tensor_max(dst, rv[:, :, :, :, 0],
                                                 rv[:, :, :, :, 1])
                if pair == 0 and half == 0:
                    nc.scalar.dma_start(cst_sb[:], cst_d)
                    nc.sync.dma_start(w2m_sb[:], w2m_d)
                    nc.scalar.dma_start(wfcb_sb[:], wfcb_d)
                # ---- T1 transposes into wide psum + relu/bias evict ----
                x2cat = []
                for ww in range(3):
                    tpw = tpw_pool.tile([120, 512], FP16, name="tpw_t",
                                        tag="tpw")
                    for q in range(2):
                        prp = prp_t[ww * 2 + q]
                        for u in range(2):
                            nc.tensor.transpose(
                                tpw[:, (q * 2 + u) * 128:
                                    (q * 2 + u + 1) * 128],
                                prp[:, u * 120:u * 120 + 120], identb[:])
                    x2c = x2_pool.tile([120, 512], FP16, name="x2c_t",
                                       tag=f"x2c{ww}")
                    relu_bias_evict(ww + blk, x2c[:], tpw[:],
                                    b1_sb[:, 0:1], 512)
                    x2cat.append(x2c)
                # ---- conv2 + pool2 + T2 + evict ----
                tp2w = tpw_pool.tile([80, 512], FP16, name="tp2w_t", tag="tpw")
                for g in range(4):
                    ps2g = ps2_pool.tile([128, 320], F32,
                                         name=f"ps2_{g}", tag="ps2")
                    for d in range(6):
                        r = 2 * g + d
                        lhsT = x2cat[r // 4][:, (r % 4) * 128:
                                             (r % 4 + 1) * 128]
                        nc.tensor.matmul(ps2g[:], lhsT,
                                         w2m_sb[:, (5 - d) * 160:
                                                (7 - d) * 160],
                                         start=(d == 0), stop=(d == 5))
                        if d == 5:
                            p2 = prp_pool.tile([128, 80], FP16, name="p2_t",
                                               tag="p2")
                            p2v = p2.rearrange("p (o s) -> p o s", o=20)
                            if g % 2 == 0:
                                src = ps2g.rearrange(
                                    "p (pl o s tc) -> p o s pl tc",
                                    pl=2, o=20, s=4)
                                nc.vector.reduce_max(p2v, src, axis=AX.XY)
                            else:
                                tmp2 = tmp_pool.tile([128, 320], FP16,
                                                     name="tmp2_t", tag="tmp2")
                                nc.scalar.copy(tmp2[:], ps2g[:])
                                t2v = tmp2.rearrange("p (pl c) -> p pl c",
                                                     pl=2)
                                rm2 = tmp_pool.tile([128, 160], FP16,
                                                    name="rm2_t", tag="rm2")
                                nc.vector.tensor_max(rm2[:], t2v[:, 0],
                                                     t2v[:, 1])
                                r2v = rm2.rearrange(
                                    "p (o s tc) -> p o s tc", o=20, s=4)
                                nc.vector.tensor_max(p2v, r2v[:, :, :, 0],
                                                     r2v[:, :, :, 1])
                            nc.tensor.transpose(
                                tp2w[:, g * 128:(g + 1) * 128], p2[:],
                                identb[:])
                relu_bias_evict(blk, f_all[:, half * 512:half * 512 + 512],
                                tp2w[:], b2_sb[:, 0:1], 512)
            # ---- fc1 over the 256-sample pair ----
            psf1 = psf_pool.tile([50, 256], F32, name="psf1", tag="psf")
            for g in range(4):
                nc.tensor.matmul(psf1[:], wfc1_sb[:, g * 50:(g + 1) * 50],
                                 fview[:, g], start=(g == 0), stop=(g == 3))
            fc1o = fc1o_pool.tile([50, 256], FP16, name="fc1o", tag="fc1o")
            nc.scalar.activation(fc1o[:], psf1[:], AF.Relu,
                                 bias=bf1_sb[:, 0:1])
            # ---- fc2 + stabilized shift (log_softmax epilogue is batched) --
            for half in range(2):
                blk = pair * 2 + half
                psf2 = psf_pool.tile([128, 10], F32, name="psf2", tag="psf")
                nc.tensor.matmul(psf2[:],
                                 fc1o[:, half * 128:half * 128 + 128],
                                 wfc2_sb[:], start=True, stop=True)
                # t1 = psf2 + (fc2_b - 10): a constant shift is exact for
                # log_softmax and keeps exp() comfortably in fp32 range
                nc.vector.tensor_add(t1_all[:, blk * 10:blk * 10 + 10],
                                     psf2[:], bc2_sb[:])
            # ---- batched log_softmax epilogue, one half-core at a time ----
            if pair % (max(n_pair // 2, 1)) == max(n_pair // 2, 1) - 1:
                hb = 2 * (pair + 1 - max(n_pair // 2, 1))  # first blk of half
                nb = 2 * max(n_pair // 2, 1)
                c0 = hb * 10
                tslice = t1_all[:, c0:c0 + 10 * nb]
                e_all = sm_pool.tile([128, 10 * nb], F32, name="e_all",
                                     tag="e_all")
                nc.scalar.activation(e_all[:], tslice, AF.Exp)
                se = sm_pool.tile([128, nb], F32, name="se", tag="se")
                nc.vector.reduce_sum(
                    se[:], e_all.rearrange("p (b t) -> p b t", t=10),
                    axis=AX.X)
                ls = sm_pool.tile([128, nb], F32, name="ls", tag="ls")
                nc.scalar.activation(ls[:], se[:], AF.Ln)
                yo = sm_pool.tile([128, 10 * nb], F32, name="yo", tag="yo")
                for b in range(nb):
                    nc.vector.tensor_scalar_sub(
                        yo[:, b * 10:b * 10 + 10],
                        t1_all[:, (hb + b) * 10:(hb + b) * 10 + 10],
                        ls[:, b:b + 1])
                nc.scalar.dma_start(
                    y[hb * 128:(hb + nb) * 128]
                    .rearrange("(blk p) c -> p blk c", p=128),
                    yo.rearrange("p (blk c) -> p blk c", c=10))

    nc.compile()
    return nc


_PROGRAM_CACHE = {}


def _get_program(b_core):
    if b_core not in _PROGRAM_CACHE:
        _PROGRAM_CACHE[b_core] = _build(b_core)
    return _PROGRAM_CACHE[b_core]


def make_in_maps(x, weights, b_core=B_CORE, n_cores=N_CORES):
    """Shard x over cores; replicate the (rearranged) parameters."""
    f32 = np.float32
    xr = np.asarray(x, dtype=f32).reshape(-1, 28, 28)
    in_maps = []
    for c in range(n_cores):
        xc = xr[c * b_core:(c + 1) * b_core]  # [b_core, 28, 28]
        xwin = np.empty((12, 128, b_core), np.float16)
        for w in range(6):
            for h in range(2):
                win = xc[:, 4 * w:4 * w + 8, 12 * h:12 * h + 16]
                xwin[w * 2 + h] = win.reshape(b_core, 128).T
        m = {"xw": np.ascontiguousarray(xwin)}
        m.update(weights)
        in_maps.append(m)
    return in_maps


def kernel(**inputs):
    x = np.asarray(inputs["x"], dtype=np.float32)
    weights = _prep_weights(
        np.asarray(inputs["mask_w"], np.float32),
        np.asarray(inputs["conv1_w"], np.float32),
        np.asarray(inputs["conv1_b"], np.float32),
        np.asarray(inputs["conv2_w"], np.float32),
        np.asarray(inputs["conv2_b"], np.float32),
        np.asarray(inputs["fc1_w"], np.float32),
        np.asarray(inputs["fc1_b"], np.float32),
        np.asarray(inputs["fc2_w"], np.float32),
        np.asarray(inputs["fc2_b"], np.float32),
    )
    nc = _get_program(B_CORE)
    in_maps = make_in_maps(x, weights)
    res = run_bass_kernel_spmd(nc, in_maps, list(range(N_CORES)))
    out = np.concatenate([res.results[c]["y"] for c in range(N_CORES)], axis=0)
    return np.ascontiguousarray(out.astype(np.float32))


if __name__ == "__main__":
    rng = np.random.default_rng(0)
    ins = {
        "x": rng.standard_normal((B_TOTAL, 1, 28, 28), dtype=np.float32),
        "mask_w": rng.standard_normal((28, 28), dtype=np.float32) * 0.1,
        "conv1_w": rng.standard_normal((10, 1, 5, 5), dtype=np.float32) * 0.2,
        "conv1_b": rng.standard_normal((10,), dtype=np.float32) * 0.1,
        "conv2_w": rng.standard_normal((20, 10, 5, 5), dtype=np.float32) * 0.06,
        "conv2_b": rng.standard_normal((20,), dtype=np.float32) * 0.1,
        "fc1_w": rng.standard_normal((50, 320), dtype=np.float32) * 0.05,
        "fc1_b": rng.standard_normal((50,), dtype=np.float32) * 0.1,
        "fc2_w": rng.standard_normal((10, 50), dtype=np.float32) * 0.14,
        "fc2_b": rng.standard_normal((10,), dtype=np.float32) * 0.1,
    }
    out = kernel(**ins)
    print(out.shape, out.dtype, out[:2])



# revision 5
# speedup vs baseline: 1.2373x; 1.2373x over previous
"""Trainium2 Bass kernel for nn_CNN_Casual (LeNet-ish CNN, B=8192), v2.

Pure data parallel over 8 NeuronCores: 1024 samples per core, parameters
replicated, one SPMD Bass program.

Design notes (cost model: matmul = out_free x 0.42ns x n_acc; DVE 2x for
fp16 ops (4x for tensor_scalar, all-SBUF); Act 0.83ns/elem, no fast modes;
GpSimd 1.39ns/elem + 95ns launch; Ldweights free):

  conv1  : weights-stationary. Stationary = masked Toeplitz block
           [128 = 8x16 input window, 120 = (P rowpair, o ch, Q colpair)],
           moving = xw window data [128, 512 samples]. 12 windows x 4
           (m rowparity, qp colparity) x 2 chunks = 96 matmuls into 4-bank
           PSUM mega tiles [120, (m, qp, 512)] fp32. No transposes needed:
           output is already [features, samples].
  pool1  : s1 = DVE TT max over qp pairs (cross-bank strided views, one op
           covers both m groups), s2 = GpSimd TT max over m pairs (SBUF
           fp16), then one DVE tensor_scalar (4x mode) applies conv1 bias +
           relu. Partition-remapping SBUF->SBUF DMAs scatter the result
           [120 = (P, o, Q), 1024] into per-row conv2 stationary tiles
           x2[r=2w+P] with layout [(h, o, Q), 1024]. A few (w, h) groups are
           instead drained by Act copies + fp16 TTs (engine balance).
  conv2  : data-stationary: stationary = x2[r] sample-slice [120, 128],
           moving = weight block W_ki [120, 160 = (o2, s2, tc)]; output rows
           2g / 2g+1 are two independent 160-wide 5-step accumulation chains
           in one [128, 320] fp32 PSUM tile (no zero blocks).
  pool2  : s1 = DVE TT max over tc pairs (strided PSUM views) -> fp16, s2 =
           GpSimd TT max over pl pairs -> [128, 80 = (o2, s2)]; PE transpose
           into [80, 512] fp16 PSUM; Act evicts with relu + conv2 bias.
  fc1    : stationary wfc1_g [80, 50] x 4 groups accumulate over moving
           f_g [80, 512]; Act relu+bias evict -> fc1o [50, 1024] fp16.
  fc2    : stationary fc1o [50, 128] per block, moving wfc2 [50, 10].
  softmax: per block t1 = psf2 + (fc2_b - 10) (constant shift is exact for
           log_softmax); batched half-core epilogue: one Exp, windowed
           reduce_sum, one Ln, per-block subtract; DMA out.

The whole thing is software-pipelined over conv1 windows w = 0..5 with
conv2 group g = w - 2 interleaved in emission order so PE never stalls on
PSUM reuse. dtypes: fp16 matmul operands / pooled activations (|x| <= ~30,
end-to-end max rel err ~5e-4 vs fp32 reference); PSUM always fp32 except
transpose tiles; pooling stage 1 reads PSUM fp32.
"""

from contextlib import ExitStack

import numpy as np

import concourse.mybir as mybir
import concourse.tile as tile
from concourse import bacc
from concourse.bass_utils import run_bass_kernel_spmd

F32 = mybir.dt.float32
FP16 = mybir.dt.float16
AF = mybir.ActivationFunctionType
AX = mybir.AxisListType

N_CORES = 8
B_TOTAL = 8192
B_CORE = B_TOTAL // N_CORES  # 1024

# engine-balance knobs
# every Nth conv1 wave sends one duo through the Act-copy drain path
WAVE_ACT_EVERY = 1
# conv2 (g, b) pool2 drain via Act copy instead of a DVE TensorReduce
# (off for the tail half-group so the Act queue stays clear for Exp/Ln)
ASSIST_P2 = lambda g, b: b % 4 != 0 and not (g == 3 and b >= 4)


# --------------------------------------------------------------------------
# Host-side weight preparation (tiny tensors; exact rearrangement only)
# --------------------------------------------------------------------------
def _prep_weights(mask_w, conv1_w, conv1_b, conv2_w, conv2_b, fc1_w, fc1_b,
                  fc2_w, fc2_b):
    f32 = np.float32
    sig = (1.0 / (1.0 + np.exp(-mask_w.astype(f32)))).astype(f32)  # [28,28]

    # conv1 Toeplitz, weights-stationary, mask folded in.
    # block t=(w,h), sub-block (m, qp): [128 = (il, jl), 124 = (slot, o, Q)]
    # out (p = 4w + 2P + m, q = 12h + 2Q + qp); partition slot (P ^ h) * 64
    # (engines cannot remap partitions, so the slot choice makes every
    # pool1-finish write land at its source partition offset);
    # K row (il = 2P + m + ki, jl = 2Q + qp + kj)
    w1m = np.zeros((128, 5952), f32)
    for w in range(6):
        for h in range(2):
            t = 2 * w + h
            for m in range(2):
                for qp in range(2):
                    blk = t * 496 + (2 * m + qp) * 124
                    for P in range(2):
                        for o in range(10):
                            for Q in range(6):
                                col = blk + (P ^ h) * 64 + o * 6 + Q
                                for ki in range(5):
                                    il = 2 * P + m + ki
                                    for kj in range(5):
                                        jl = 2 * Q + qp + kj
                                        w1m[il * 16 + jl, col] = (
                                            conv1_w[o, 0, ki, kj]
                                            * sig[4 * w + il, 12 * h + jl])
    w1m = w1m.astype(np.float16)

    # conv2 blocks per (row parity rho, ki): [124 = (slot, c, Q), 160 =
    # (o2, s2, tc)]; x2[r] partition slot s holds pooled col-half
    # hh = s ^ (r % 2); row = (c, j' = 6*hh + Q); col = output
    # (o2, q2 = 2s2 + tc); nonzero where kj = j' - q2 in [0, 5)
    w2m = np.zeros((2, 5, 124, 160), f32)
    for rho in range(2):
        for sl in range(2):
            hh = sl ^ rho
            for c in range(10):
                for Q in range(6):
                    r = sl * 64 + c * 6 + Q
                    jp = 6 * hh + Q
                    for o2 in range(20):
                        for s2 in range(4):
                            for tc in range(2):
                                q2 = 2 * s2 + tc
                                kj = jp - q2
                                if 0 <= kj < 5:
                                    w2m[rho, :, r, o2 * 8 + s2 * 2 + tc] = \
                                        conv2_w[o2, c, :, kj]
    w2m = np.ascontiguousarray(
        w2m.transpose(2, 0, 1, 3).reshape(124, 1600)).astype(np.float16)

    # fc1 weights per pooled-row group g: rows (o2, s2), torch flatten order
    # of the conv2 activations is (o2, g, s2).
    fc1w4 = fc1_w.reshape(50, 20, 4, 4)  # [m, o2, g, s2]
    wfc1 = np.concatenate(
        [np.ascontiguousarray(fc1w4[:, :, g, :].reshape(50, 80).T)
         for g in range(4)],
        axis=1,
    )  # [80, 200]

    # const blob 1 (fp32): bc2 | b1 | b2 | bf1  -> [128, 13]
    cst = np.zeros((128, 13), f32)
    cst[:, 0:10] = np.tile(fc2_b.astype(f32).reshape(1, 10) - 10.0, (128, 1))
    b1v = np.repeat(conv1_b.astype(f32), 6)
    cst[0:60, 10] = b1v
    cst[64:124, 10] = b1v
    cst[0:80, 11] = np.repeat(conv2_b.astype(f32), 4)
    cst[0:50, 12] = fc1_b.astype(f32)

    # const blob 2 (fp16): fc2_w.T | wfc1 -> [80, 210]
    wfcb = np.zeros((80, 210), np.float16)
    wfcb[0:50, 0:10] = fc2_w.T.astype(np.float16)
    wfcb[:, 10:210] = wfc1.astype(np.float16)

    idb = np.eye(128).astype(np.float16)
    return dict(w1m=w1m, w2m=w2m, wfcb=wfcb, cst=cst, idb=idb)


# --------------------------------------------------------------------------
# Device program
# --------------------------------------------------------------------------
def _build(b_core):
    assert b_core == 1024
    n_blk = b_core // 128  # 8

    nc = bacc.Bacc("TRN2", target_bir_lowering=False, debug=False,
                   num_devices=N_CORES)

    xw_d = nc.dram_tensor("xw", [12, 128, b_core], FP16,
                          kind="ExternalInput").ap()
    w1m_d = nc.dram_tensor("w1m", [128, 5952], FP16,
                           kind="ExternalInput").ap()
    w2m_d = nc.dram_tensor("w2m", [124, 1600], FP16, kind="ExternalInput").ap()
    wfcb_d = nc.dram_tensor("wfcb", [80, 210], FP16, kind="ExternalInput").ap()
    cst_d = nc.dram_tensor("cst", [128, 13], F32, kind="ExternalInput").ap()
    idb_d = nc.dram_tensor("idb", [128, 128], FP16, kind="ExternalInput").ap()
    y = nc.dram_tensor("y", [b_core, 10], F32, kind="ExternalOutput").ap()

    MAXO = mybir.AluOpType.max
    ADD = mybir.AluOpType.add

    with tile.TileContext(nc) as tc, ExitStack() as ctx:
        consts = ctx.enter_context(tc.tile_pool(name="consts", bufs=1))
        identb = consts.tile([128, 128], FP16)
        w1m_sb = consts.tile([128, 5952], FP16)
        w2m_sb = consts.tile([124, 1600], FP16)
        wfcb_sb = consts.tile([80, 210], FP16)
        cst_sb = consts.tile([128, 13], F32)

        bc2_sb = cst_sb[:, 0:10]
        b1_sb = cst_sb[0:124, 10:11]
        b2_sb = cst_sb[0:80, 11:12]
        bf1_sb = cst_sb[0:50, 12:13]
        wfc2_sb = wfcb_sb[0:50, 0:10]
        wfc1_sb = wfcb_sb[:, 10:210]

        # xw input: one big tile, window t at free offset t*b_core
        xw_all = consts.tile([128, 12 * b_core], FP16, name="xw_all")
        # conv2 stationary tiles, one per pooled row r
        x2 = [consts.tile([124, b_core], FP16, name=f"x2_{r}")
              for r in range(12)]
        # fc1 moving tiles, one per conv2 row-pair group g
        f_g = [consts.tile([80, b_core], FP16, name=f"f{g}")
               for g in range(4)]
        fc1o = consts.tile([50, b_core], FP16, name="fc1o")
        t1_all = consts.tile([128, 10 * n_blk], F32)

        # Bulk DMA all goes through the otherwise-idle SP queue, in few big
        # transfers (each dma_start costs ~630ns on the serial HWDGE): xw
        # windows in 4 staged chunks, w1m in 2, so the first conv1 matmul
        # only waits for chunk one.
        def prefetch_xw(tlo, thi):
            nc.sync.dma_start(
                xw_all[:, tlo * b_core:thi * b_core]
                .rearrange("p (t n) -> p t n", n=b_core),
                xw_d[tlo:thi].rearrange("t p n -> p t n"))

        prefetch_xw(0, 1)
        nc.sync.dma_start(w1m_sb[:, 0:992], w1m_d[:, 0:992])
        prefetch_xw(1, 3)
        nc.sync.dma_start(w1m_sb[:, 992:2976], w1m_d[:, 992:2976])
        nc.sync.dma_start(cst_sb[:], cst_d)
        prefetch_xw(3, 6)
        nc.sync.dma_start(w2m_sb[:], w2m_d)
        nc.sync.dma_start(wfcb_sb[:], wfcb_d)
        nc.sync.dma_start(identb[:], idb_d)
        prefetch_xw(6, 9)
        nc.sync.dma_start(w1m_sb[:, 2976:5952], w1m_d[:, 2976:5952])
        prefetch_xw(9, 12)

        ps1_pool = ctx.enter_context(tc.tile_pool(name="ps1", bufs=2,
                                                  space="PSUM"))
        ps2_pool = ctx.enter_context(tc.tile_pool(name="ps2", bufs=2,
                                                  space="PSUM"))
        tpf_pool = ctx.enter_context(tc.tile_pool(name="tpf", bufs=2,
                                                  space="PSUM"))
        u_pool = ctx.enter_context(tc.tile_pool(name="u", bufs=3))
        v_pool = ctx.enter_context(tc.tile_pool(name="v", bufs=2))
        p2_pool = ctx.enter_context(tc.tile_pool(name="p2", bufs=5))
        sm_pool = ctx.enter_context(tc.tile_pool(name="sm", bufs=3))

        def conv1_wave(w, h, c, slots):
            """4 matmuls for window t=(w,h), chunk c, into 4 psum slots."""
            t = 2 * w + h
            mov = xw_all[:, t * b_core + c * 512:t * b_core + (c + 1) * 512]
            for i, (m, qp) in enumerate(((0, 0), (0, 1), (1, 0), (1, 1))):
                blk = t * 496 + (2 * m + qp) * 124
                nc.tensor.matmul(
                    slots[i], w1m_sb[:, blk:blk + 124], mov,
                    start=True, stop=True)

        wave_seq = [0]

        def emit_wave(w, h, c, startup):
            """conv1 wave into 4 one-bank psum slots, pool1 drain, then
            bias+relu straight into the x2 chunk halves (partition remap).

            During phase A (pure conv1, no conv2 to fill PE stalls)
            alternate waves borrow the idle ps2/tpf banks so the pipeline
            is two waves deep.
            """
            wave_seq[0] += 1
            duo0 = ps1_pool.tile([124, 1024], F32, name="duo0", tag="mega")
            duo1 = ps1_pool.tile([124, 1024], F32, name="duo1", tag="mega")
            duos = [duo0, duo1]
            slots = [duo0[:, 0:512], duo0[:, 512:1024],
                     duo1[:, 0:512], duo1[:, 512:1024]]
            conv1_wave(w, h, c, slots)
            # legal psum drains (at most ONE psum input per instruction):
            # DVE TensorReduce over a duo's qp pair, or Act copy to fp16
            # followed by a GpSimd TT max, per the balance knobs.
            u = u_pool.tile([124, 1024], FP16, name="u", tag="u")
            uv = u.rearrange("p (m n) -> p m n", m=2)
            act_duos = 1 if wave_seq[0] % WAVE_ACT_EVERY == 0 else 0
            for m in range(2):
                if m >= 2 - act_duos:
                    cpy = u_pool.tile([124, 1024], FP16, name="cpy",
                                      tag="cpy")
                    cv = cpy.rearrange("p (q n) -> p q n", q=2)
                    nc.scalar.copy(cv[:], duos[m].rearrange(
                        "p (q n) -> p q n", q=2))
                    nc.vector.tensor_tensor(uv[:, m], cv[:, 0], cv[:, 1],
                                            op=MAXO)
                else:
                    dm = duos[m].rearrange("p (q n) -> p n q", q=2)
                    nc.vector.reduce_max(uv[:, m], dm, axis=AX.X)
            vc = v_pool.tile([124, 512], FP16, name="vc", tag="v")
            nc.vector.tensor_tensor(vc[:], uv[:, 0], uv[:, 1], op=MAXO)
            cols = slice(c * 512, (c + 1) * 512)
            for sl in range(2):
                row = 2 * w + (sl ^ h)
                pr = slice(sl * 64, sl * 64 + 60)
                nc.vector.tensor_scalar(x2[row][pr, cols], vc[pr, :],
                                        b1_sb[pr, 0:1], 0.0,
                                        op0=ADD, op1=MAXO)

        def conv2_block(g, b):
            """conv2 + pool2 for row-pair g, sample block b -> [128,80]."""
            ps2 = ps2_pool.tile([128, 320], F32, name="ps2", tag="ps2")
            for half in range(2):  # pl = half: out row 2g + half
                r0 = 2 * g + half
                for ki in range(5):
                    rho = (r0 + ki) % 2
                    blk = (rho * 5 + ki) * 160
                    nc.tensor.matmul(
                        ps2[:, half * 160:(half + 1) * 160],
                        x2[r0 + ki][:, b * 128:(b + 1) * 128],
                        w2m_sb[:, blk:blk + 160],
                        start=(ki == 0), stop=(ki == 4))
            # pool2: 4:1 over (pl, tc); legal single-psum-input drains only
            pg = p2_pool.tile([128, 80], FP16, name="pg", tag="pg")
            if ASSIST_P2(g, b):
                # Act drains to fp16, maxes on GpSimd + DVE
                cp2 = p2_pool.tile([128, 320], FP16, name="cp2", tag="cp2")
                nc.scalar.copy(cp2[:], ps2[:])
                cpv = cp2.rearrange("p (pl o s tc) -> p pl o s tc",
                                    pl=2, o=20, s=4)
                p2a = p2_pool.tile([128, 160], FP16, name="p2a", tag="p2a")
                av = p2a.rearrange("p (pl c) -> p pl c", pl=2)
                nc.vector.tensor_tensor(av[:], cpv[:, :, :, :, 0],
                                        cpv[:, :, :, :, 1], op=MAXO)
                nc.vector.tensor_tensor(pg[:], av[:, 0], av[:, 1], op=MAXO)
            else:
                # one DVE 6D TensorReduce does the whole 4:1
                pv = ps2.rearrange("p (pl o s tc) -> p o s pl tc", pl=2,
                                   o=20, s=4)
                nc.vector.reduce_max(pg.rearrange("p (o s) -> p o s", o=20),
                                     pv, axis=AX.XY)
            return pg

        conv2_state = {"pending": None}

        def flush_transpose():
            """Emit the delayed transpose (and evict on the 4th of a half)."""
            if conv2_state["pending"] is None:
                return
            g, b, pg = conv2_state["pending"]
            conv2_state["pending"] = None
            bh, i = divmod(b, 4)
            if i == 0:
                conv2_state["tp2"] = tpf_pool.tile([80, 512], FP16,
                                                   name="tp2", tag="tpf")
            tp2 = conv2_state["tp2"]
            nc.tensor.transpose(tp2[:, i * 128:(i + 1) * 128], pg[:],
                                identb[:])
            if i == 3:
                dst = f_g[g][:, bh * 512:(bh + 1) * 512]
                if g == 3 and bh == 1:
                    # tail: keep the Act queue free for Exp/Ln table loads
                    nc.vector.tensor_scalar(dst, tp2[:], b2_sb[:, 0:1],
                                            0.0, op0=ADD, op1=MAXO)
                else:
                    nc.scalar.activation(dst, tp2[:], AF.Relu,
                                         bias=b2_sb[:, 0:1])

        def emit_conv2_block(g, b):
            """One conv2 block; its transpose is delayed one block so the
            PE never waits on the DVE->GpSimd pool2 chain."""
            pg = conv2_block(g, b)
            flush_transpose()
            conv2_state["pending"] = (g, b, pg)

        se_all = consts.tile([128, 8], F32, name="se_all")
        yo_all = consts.tile([128, 80], F32, name="yo_all")

        def emit_fc1(cc):
            psf1 = tpf_pool.tile([50, 512], F32, name="psf1", tag="tpf")
            for g in range(4):
                nc.tensor.matmul(psf1[:], wfc1_sb[:, g * 50:(g + 1) * 50],
                                 f_g[g][:, cc * 512:(cc + 1) * 512],
                                 start=(g == 0), stop=(g == 3))
            dst = fc1o[:, cc * 512:(cc + 1) * 512]
            if cc == 1:
                nc.vector.tensor_scalar(dst, psf1[:], bf1_sb[:, 0:1], 0.0,
                                        op0=ADD, op1=MAXO)
            else:
                nc.scalar.activation(dst, psf1[:], AF.Relu,
                                     bias=bf1_sb[:, 0:1])

        def emit_fc2_block(b):
            psf2 = tpf_pool.tile([128, 10], F32, name="psf2", tag="tpf")
            nc.tensor.matmul(psf2[:], fc1o[:, b * 128:(b + 1) * 128],
                             wfc2_sb[:], start=True, stop=True)
            nc.vector.tensor_add(t1_all[:, b * 10:b * 10 + 10],
                                 psf2[:], bc2_sb[:])
            if b % 4 == 3:
                # Exp + per-block sums now; Ln/sub/output deferred to the end
                hb = b - 3
                tslice = t1_all[:, hb * 10:hb * 10 + 40]
                e_all = sm_pool.tile([128, 40], F32, name="e_all",
                                     tag="e_all")
                nc.scalar.activation(e_all[:], tslice, AF.Exp)
                nc.vector.reduce_sum(
                    se_all[:, hb:hb + 4],
                    e_all.rearrange("p (b t) -> p b t", t=10), axis=AX.X)

        def emit_epilogue():
            ls = sm_pool.tile([128, 8], F32, name="ls", tag="ls")
            nc.scalar.activation(ls[:], se_all[:], AF.Ln)
            for k in range(n_blk):
                nc.vector.tensor_scalar_sub(
                    yo_all[:, k * 10:k * 10 + 10],
                    t1_all[:, k * 10:k * 10 + 10], ls[:, k:k + 1])
            nc.sync.dma_start(
                y.rearrange("(blk p) c -> p blk c", p=128),
                yo_all.rearrange("p (blk c) -> p blk c", c=10))

        # ---- software pipeline ----
        # Phase A: c=0 waves of w=0..2 (pure conv1, two-deep psum
        # ping-pong). Phase B: c=1 waves of w=0..2 with the first conv2
        # blocks of g=0 (their sample blocks only need the c=0 chunk).
        # Phase C: w=3..5, each wave slot paired with two conv2 blocks of
        # g=w-3 (b4..7) then g=w-2 (b0..3). Tail: g=3 b4..7 + fc + softmax.
        for w in range(3):
            for h in range(2):
                emit_wave(w, h, 0, "A")
        for w in range(3):
            for h in range(2):
                emit_wave(w, h, 1, "B")
                k = 2 * w + h
                if k >= 2:
                    emit_conv2_block(0, k - 2)
        for w in range(3, 6):
            for h in range(2):
                for c in range(2):
                    emit_wave(w, h, c, None)
                    slot = 2 * h + c
                    if slot < 2:
                        emit_conv2_block(w - 3, 4 + slot * 2)
                        emit_conv2_block(w - 3, 5 + slot * 2)
                    else:
                        emit_conv2_block(w - 2, (slot - 2) * 2)
                        emit_conv2_block(w - 2, (slot - 2) * 2 + 1)

        # tail: last conv2 half-group + fc + epilogue, overlapped
        flush_transpose()
        emit_fc1(0)
        emit_conv2_block(3, 4)
        emit_conv2_block(3, 5)
        for b in range(4):
            emit_fc2_block(b)
        emit_conv2_block(3, 6)
        emit_conv2_block(3, 7)
        flush_transpose()
        emit_fc1(1)
        for b in range(4, n_blk):
            emit_fc2_block(b)
        emit_epilogue()

    nc.compile()
    return nc


_PROGRAM_CACHE = {}


def _get_program(b_core):
    if b_core not in _PROGRAM_CACHE:
        _PROGRAM_CACHE[b_core] = _build(b_core)
    return _PROGRAM_CACHE[b_core]


def make_in_maps(x, weights, b_core=B_CORE, n_cores=N_CORES):
    """Shard x over cores; replicate the (rearranged) parameters."""
    f32 = np.float32
    xr = np.asarray(x, dtype=f32).reshape(-1, 28, 28)
    in_maps = []
    for c in range(n_cores):
        xc = xr[c * b_core:(c + 1) * b_core]  # [b_core, 28, 28]
        xwin = np.empty((12, 128, b_core), np.float16)
        for w in range(6):
            for h in range(2):
                win = xc[:, 4 * w:4 * w + 8, 12 * h:12 * h + 16]
                xwin[w * 2 + h] = win.reshape(b_core, 128).T
        m = {"xw": np.ascontiguousarray(xwin)}
        m.update(weights)
        in_maps.append(m)
    return in_maps


def kernel(**inputs):
    x = np.asarray(inputs["x"], dtype=np.float32)
    weights = _prep_weights(
        np.asarray(inputs["mask_w"], np.float32),
        np.asarray(inputs["conv1_w"], np.float32),
        np.asarray(inputs["conv1_b"], np.float32),
        np.asarray(inputs["conv2_w"], np.float32),
        np.asarray(inputs["conv2_b"], np.float32),
        np.asarray(inputs["fc1_w"], np.float32),
        np.asarray(inputs["fc1_b"], np.float32),
        np.asarray(inputs["fc2_w"], np.float32),
        np.asarray(inputs["fc2_b"], np.float32),
    )
    nc = _get_program(B_CORE)
    in_maps = make_in_maps(x, weights)
    res = run_bass_kernel_spmd(nc, in_maps, list(range(N_CORES)))
    out = np.concatenate([res.results[c]["y"] for c in range(N_CORES)], axis=0)
    return np.ascontiguousarray(out.astype(np.float32))


if __name__ == "__main__":
    rng = np.random.default_rng(0)
    ins = {
        "x": rng.standard_normal((B_TOTAL, 1, 28, 28), dtype=np.float32),
        "mask_w": rng.standard_normal((28, 28), dtype=np.float32) * 0.1,
        "conv1_w": rng.standard_normal((10, 1, 5, 5), dtype=np.float32) * 0.2,
        "conv1_b": rng.standard_normal((10,), dtype=np.float32) * 0.1,
        "conv2_w": rng.standard_normal((20, 10, 5, 5), dtype=np.float32) * 0.06,
        "conv2_b": rng.standard_normal((20,), dtype=np.float32) * 0.1,
        "fc1_w": rng.standard_normal((50, 320), dtype=np.float32) * 0.05,
        "fc1_b": rng.standard_normal((50,), dtype=np.float32) * 0.1,
        "fc2_w": rng.standard_normal((10, 50), dtype=np.float32) * 0.14,
        "fc2_b": rng.standard_normal((10,), dtype=np.float32) * 0.1,
    }
    out = kernel(**ins)
    print(out.shape, out.dtype, out[:2])


# revision 6
# speedup vs baseline: 1.3341x; 1.0783x over previous
"""Trainium2 Bass kernel for nn_CNN_Casual (LeNet-ish CNN, B=8192), v2.

Pure data parallel over 8 NeuronCores: 1024 samples per core, parameters
replicated, one SPMD Bass program.

Design notes (cost model: matmul = out_free x 0.42ns x n_acc; DVE 2x for
fp16 ops (4x for tensor_scalar, all-SBUF); Act 0.83ns/elem, no fast modes;
GpSimd 1.39ns/elem + 95ns launch; Ldweights free):

  conv1  : weights-stationary. Stationary = masked Toeplitz block
           [128 = 8x16 input window, 120 = (P rowpair, o ch, Q colpair)],
           moving = xw window data [128, 512 samples]. 12 windows x 4
           (m rowparity, qp colparity) x 2 chunks = 96 matmuls into 4-bank
           PSUM mega tiles [120, (m, qp, 512)] fp32. No transposes needed:
           output is already [features, samples].
  pool1  : s1 = DVE TT max over qp pairs (cross-bank strided views, one op
           covers both m groups), s2 = GpSimd TT max over m pairs (SBUF
           fp16), then one DVE tensor_scalar (4x mode) applies conv1 bias +
           relu. Partition-remapping SBUF->SBUF DMAs scatter the result
           [120 = (P, o, Q), 1024] into per-row conv2 stationary tiles
           x2[r=2w+P] with layout [(h, o, Q), 1024]. A few (w, h) groups are
           instead drained by Act copies + fp16 TTs (engine balance).
  conv2  : data-stationary: stationary = x2[r] sample-slice [120, 128],
           moving = weight block W_ki [120, 160 = (o2, s2, tc)]; output rows
           2g / 2g+1 are two independent 160-wide 5-step accumulation chains
           in one [128, 320] fp32 PSUM tile (no zero blocks).
  pool2  : s1 = DVE TT max over tc pairs (strided PSUM views) -> fp16, s2 =
           GpSimd TT max over pl pairs -> [128, 80 = (o2, s2)]; PE transpose
           into [80, 512] fp16 PSUM; Act evicts with relu + conv2 bias.
  fc1    : stationary wfc1_g [80, 50] x 4 groups accumulate over moving
           f_g [80, 512]; Act relu+bias evict -> fc1o [50, 1024] fp16.
  fc2    : stationary fc1o [50, 128] per block, moving wfc2 [50, 10].
  softmax: per block t1 = psf2 + (fc2_b - 10) (constant shift is exact for
           log_softmax); batched half-core epilogue: one Exp, windowed
           reduce_sum, one Ln, per-block subtract; DMA out.

The whole thing is software-pipelined over conv1 windows w = 0..5 with
conv2 group g = w - 2 interleaved in emission order so PE never stalls on
PSUM reuse. dtypes: fp16 matmul operands / pooled activations (|x| <= ~30,
end-to-end max rel err ~5e-4 vs fp32 reference); PSUM always fp32 except
transpose tiles; pooling stage 1 reads PSUM fp32.
"""

from contextlib import ExitStack

import numpy as np

import concourse.mybir as mybir
import concourse.tile as tile
from concourse import bacc
from concourse.bass_utils import run_bass_kernel_spmd

F32 = mybir.dt.float32
FP16 = mybir.dt.float16
AF = mybir.ActivationFunctionType
AX = mybir.AxisListType

N_CORES = 8
B_TOTAL = 8192
B_CORE = B_TOTAL // N_CORES  # 1024

# engine-balance knobs
# per-wave count of duos drained via the Act-copy path (cycled)
WAVE_ACT_PATTERN = [1, 2]
# conv2 (g, b) pool2 drain via Act copy instead of a DVE TensorReduce
# (off for the tail half-group so the Act queue stays clear for Exp/Ln)
ASSIST_P2 = lambda g, b: b % 4 != 0 and not (g == 3 and b >= 4)


# --------------------------------------------------------------------------
# Host-side weight preparation (tiny tensors; exact rearrangement only)
# --------------------------------------------------------------------------
def _prep_weights(mask_w, conv1_w, conv1_b, conv2_w, conv2_b, fc1_w, fc1_b,
                  fc2_w, fc2_b):
    f32 = np.float32
    sig = (1.0 / (1.0 + np.exp(-mask_w.astype(f32)))).astype(f32)  # [28,28]

    # conv1 Toeplitz, weights-stationary, mask folded in.
    # block t=(w,h), sub-block (m, qp): [128 = (il, jl), 124 = (slot, o, Q)]
    # out (p = 4w + 2P + m, q = 12h + 2Q + qp); partition slot (P ^ h) * 64
    # (engines cannot remap partitions, so the slot choice makes every
    # pool1-finish write land at its source partition offset);
    # K row (il = 2P + m + ki, jl = 2Q + qp + kj)
    w1m = np.zeros((128, 5952), f32)
    for w in range(6):
        for h in range(2):
            t = 2 * w + h
            for m in range(2):
                for qp in range(2):
                    blk = t * 496 + (2 * m + qp) * 124
                    for P in range(2):
                        for o in range(10):
                            for Q in range(6):
                                col = blk + (P ^ h) * 64 + o * 6 + Q
                                for ki in range(5):
                                    il = 2 * P + m + ki
                                    for kj in range(5):
                                        jl = 2 * Q + qp + kj
                                        w1m[il * 16 + jl, col] = (
                                            conv1_w[o, 0, ki, kj]
                                            * sig[4 * w + il, 12 * h + jl])
    w1m = w1m.astype(np.float16)

    # conv2 blocks per (row parity rho, ki): [124 = (slot, c, Q), 160 =
    # (o2, s2, tc)]; x2[r] partition slot s holds pooled col-half
    # hh = s ^ (r % 2); row = (c, j' = 6*hh + Q); col = output
    # (o2, q2 = 2s2 + tc); nonzero where kj = j' - q2 in [0, 5)
    w2m = np.zeros((2, 5, 124, 160), f32)
    for rho in range(2):
        for sl in range(2):
            hh = sl ^ rho
            for c in range(10):
                for Q in range(6):
                    r = sl * 64 + c * 6 + Q
                    jp = 6 * hh + Q
                    for o2 in range(20):
                        for s2 in range(4):
                            for tc in range(2):
                                q2 = 2 * s2 + tc
                                kj = jp - q2
                                if 0 <= kj < 5:
                                    w2m[rho, :, r, o2 * 8 + s2 * 2 + tc] = \
                                        conv2_w[o2, c, :, kj]
    w2m = np.ascontiguousarray(
        w2m.transpose(2, 0, 1, 3).reshape(124, 1600)).astype(np.float16)

    # fc1 weights per pooled-row group g: rows (o2, s2), torch flatten order
    # of the conv2 activations is (o2, g, s2).
    fc1w4 = fc1_w.reshape(50, 20, 4, 4)  # [m, o2, g, s2]
    wfc1 = np.concatenate(
        [np.ascontiguousarray(fc1w4[:, :, g, :].reshape(50, 80).T)
         for g in range(4)],
        axis=1,
    )  # [80, 200]

    # const blob 1 (fp32): bc2 | b1 | b2 | bf1 | bc2x4 -> [128, 53]
    cst = np.zeros((128, 53), f32)
    cst[:, 0:10] = np.tile(fc2_b.astype(f32).reshape(1, 10) - 10.0, (128, 1))
    b1v = np.repeat(conv1_b.astype(f32), 6)
    cst[0:60, 10] = b1v
    cst[64:124, 10] = b1v
    cst[0:80, 11] = np.repeat(conv2_b.astype(f32), 4)
    cst[0:50, 12] = fc1_b.astype(f32)
    cst[:, 13:53] = np.tile(cst[:, 0:10], (1, 4))

    # const blob 2 (fp16): fc2_w.T | wfc1 -> [80, 210]
    wfcb = np.zeros((80, 210), np.float16)
    wfcb[0:50, 0:10] = fc2_w.T.astype(np.float16)
    wfcb[:, 10:210] = wfc1.astype(np.float16)

    idb = np.eye(128).astype(np.float16)
    return dict(w1m=w1m, w2m=w2m, wfcb=wfcb, cst=cst, idb=idb)


# --------------------------------------------------------------------------
# Device program
# --------------------------------------------------------------------------
def _build(b_core):
    assert b_core == 1024
    n_blk = b_core // 128  # 8

    nc = bacc.Bacc("TRN2", target_bir_lowering=False, debug=False,
                   num_devices=N_CORES)

    xw_d = nc.dram_tensor("xw", [12, 128, b_core], FP16,
                          kind="ExternalInput").ap()
    w1m_d = nc.dram_tensor("w1m", [128, 5952], FP16,
                           kind="ExternalInput").ap()
    w2m_d = nc.dram_tensor("w2m", [124, 1600], FP16, kind="ExternalInput").ap()
    wfcb_d = nc.dram_tensor("wfcb", [80, 210], FP16, kind="ExternalInput").ap()
    cst_d = nc.dram_tensor("cst", [128, 53], F32, kind="ExternalInput").ap()
    idb_d = nc.dram_tensor("idb", [128, 128], FP16, kind="ExternalInput").ap()
    y = nc.dram_tensor("y", [b_core, 10], F32, kind="ExternalOutput").ap()

    MAXO = mybir.AluOpType.max
    ADD = mybir.AluOpType.add

    with tile.TileContext(nc) as tc, ExitStack() as ctx:
        consts = ctx.enter_context(tc.tile_pool(name="consts", bufs=1))
        identb = consts.tile([128, 128], FP16)
        w1m_sb = consts.tile([128, 5952], FP16)
        w2m_sb = consts.tile([124, 1600], FP16)
        wfcb_sb = consts.tile([80, 210], FP16)
        cst_sb = consts.tile([128, 53], F32)

        # pre-load the one activation table that serves Copy/Relu/Exp/Ln
        # (id 6 = natural_log_exp_and_others) so the table-placement pass
        # never needs another load
        nc.scalar.add_instruction(mybir.InstLoadActFuncSet(
            name="preload_act_tbl", act_func_set_id=6, ins=[], outs=[]))

        bc2_sb = cst_sb[:, 0:10]
        bc2x4_sb = cst_sb[:, 13:53]
        b1_sb = cst_sb[0:124, 10:11]
        b2_sb = cst_sb[0:80, 11:12]
        bf1_sb = cst_sb[0:50, 12:13]
        wfc2_sb = wfcb_sb[0:50, 0:10]
        wfc1_sb = wfcb_sb[:, 10:210]

        # xw input: one big tile, window t at free offset t*b_core
        xw_all = consts.tile([128, 12 * b_core], FP16, name="xw_all")
        # conv2 stationary tiles, one per pooled row r
        x2 = [consts.tile([124, b_core], FP16, name=f"x2_{r}")
              for r in range(12)]
        # fc1 moving tiles, one per conv2 row-pair group g
        f_g = [consts.tile([80, b_core], FP16, name=f"f{g}")
               for g in range(4)]
        fc1o = consts.tile([50, b_core], FP16, name="fc1o")
        t1_all = consts.tile([128, 10 * n_blk], F32)

        # Bulk DMA all goes through the otherwise-idle SP queue, in few big
        # transfers (each dma_start costs ~630ns on the serial HWDGE): xw
        # windows in 4 staged chunks, w1m in 2, so the first conv1 matmul
        # only waits for chunk one.
        def prefetch_xw(tlo, thi):
            nc.sync.dma_start(
                xw_all[:, tlo * b_core:thi * b_core]
                .rearrange("p (t n) -> p t n", n=b_core),
                xw_d[tlo:thi].rearrange("t p n -> p t n"))

        prefetch_xw(0, 1)
        nc.sync.dma_start(w1m_sb[:, 0:992], w1m_d[:, 0:992])
        prefetch_xw(1, 3)
        nc.sync.dma_start(w1m_sb[:, 992:2976], w1m_d[:, 992:2976])
        nc.sync.dma_start(cst_sb[:], cst_d)
        prefetch_xw(3, 6)
        nc.sync.dma_start(w2m_sb[:], w2m_d)
        nc.sync.dma_start(wfcb_sb[:], wfcb_d)
        nc.sync.dma_start(identb[:], idb_d)
        prefetch_xw(6, 9)
        nc.sync.dma_start(w1m_sb[:, 2976:5952], w1m_d[:, 2976:5952])
        prefetch_xw(9, 12)

        ps1_pool = ctx.enter_context(tc.tile_pool(name="ps1", bufs=2,
                                                  space="PSUM"))
        ps2_pool = ctx.enter_context(tc.tile_pool(name="ps2", bufs=2,
                                                  space="PSUM"))
        tpf_pool = ctx.enter_context(tc.tile_pool(name="tpf", bufs=2,
                                                  space="PSUM"))
        u_pool = ctx.enter_context(tc.tile_pool(name="u", bufs=3))
        v_pool = ctx.enter_context(tc.tile_pool(name="v", bufs=2))
        p2_pool = ctx.enter_context(tc.tile_pool(name="p2", bufs=5))
        sm_pool = ctx.enter_context(tc.tile_pool(name="sm", bufs=3))

        def conv1_wave(w, h, c, slots):
            """4 matmuls for window t=(w,h), chunk c, into 4 psum slots."""
            t = 2 * w + h
            mov = xw_all[:, t * b_core + c * 512:t * b_core + (c + 1) * 512]
            for i, (m, qp) in enumerate(((0, 0), (0, 1), (1, 0), (1, 1))):
                blk = t * 496 + (2 * m + qp) * 124
                nc.tensor.matmul(
                    slots[i], w1m_sb[:, blk:blk + 124], mov,
                    start=True, stop=True)

        wave_seq = [0]

        def emit_wave(w, h, c, startup):
            """conv1 wave into 4 one-bank psum slots, pool1 drain, then
            bias+relu straight into the x2 chunk halves (partition remap).

            During phase A (pure conv1, no conv2 to fill PE stalls)
            alternate waves borrow the idle ps2/tpf banks so the pipeline
            is two waves deep.
            """
            wave_seq[0] += 1
            duo0 = ps1_pool.tile([124, 1024], F32, name="duo0", tag="mega")
            duo1 = ps1_pool.tile([124, 1024], F32, name="duo1", tag="mega")
            duos = [duo0, duo1]
            slots = [duo0[:, 0:512], duo0[:, 512:1024],
                     duo1[:, 0:512], duo1[:, 512:1024]]
            conv1_wave(w, h, c, slots)
            # legal psum drains (at most ONE psum input per instruction):
            # DVE TensorReduce over a duo's qp pair, or Act copy to fp16
            # followed by a GpSimd TT max, per the balance knobs.
            u = u_pool.tile([124, 1024], FP16, name="u", tag="u")
            uv = u.rearrange("p (m n) -> p m n", m=2)
            act_duos = WAVE_ACT_PATTERN[wave_seq[0] % len(WAVE_ACT_PATTERN)]
            for m in range(2):
                if m >= 2 - act_duos:
                    cpy = u_pool.tile([124, 1024], FP16, name="cpy",
                                      tag="cpy")
                    cv = cpy.rearrange("p (q n) -> p q n", q=2)
                    nc.scalar.copy(cv[:], duos[m].rearrange(
                        "p (q n) -> p q n", q=2))
                    nc.vector.tensor_tensor(uv[:, m], cv[:, 0], cv[:, 1],
                                            op=MAXO)
                else:
                    dm = duos[m].rearrange("p (q n) -> p n q", q=2)
                    nc.vector.reduce_max(uv[:, m], dm, axis=AX.X)
            vc = v_pool.tile([124, 512], FP16, name="vc", tag="v")
            nc.vector.tensor_tensor(vc[:], uv[:, 0], uv[:, 1], op=MAXO)
            cols = slice(c * 512, (c + 1) * 512)
            for sl in range(2):
                row = 2 * w + (sl ^ h)
                pr = slice(sl * 64, sl * 64 + 60)
                nc.vector.tensor_scalar(x2[row][pr, cols], vc[pr, :],
                                        b1_sb[pr, 0:1], 0.0,
                                        op0=ADD, op1=MAXO)

        def conv2_block(g, b):
            """conv2 + pool2 for row-pair g, sample block b -> [128,80]."""
            ps2 = ps2_pool.tile([128, 320], F32, name="ps2", tag="ps2")
            for half in range(2):  # pl = half: out row 2g + half
                r0 = 2 * g + half
                for ki in range(5):
                    rho = (r0 + ki) % 2
                    blk = (rho * 5 + ki) * 160
                    nc.tensor.matmul(
                        ps2[:, half * 160:(half + 1) * 160],
                        x2[r0 + ki][:, b * 128:(b + 1) * 128],
                        w2m_sb[:, blk:blk + 160],
                        start=(ki == 0), stop=(ki == 4))
            # pool2: 4:1 over (pl, tc); legal single-psum-input drains only
            pg = p2_pool.tile([128, 80], FP16, name="pg", tag="pg")
            if ASSIST_P2(g, b):
                # Act drains to fp16, maxes on GpSimd + DVE
                cp2 = p2_pool.tile([128, 320], FP16, name="cp2", tag="cp2")
                nc.scalar.copy(cp2[:], ps2[:])
                cpv = cp2.rearrange("p (pl o s tc) -> p pl o s tc",
                                    pl=2, o=20, s=4)
                p2a = p2_pool.tile([128, 160], FP16, name="p2a", tag="p2a")
                av = p2a.rearrange("p (pl c) -> p pl c", pl=2)
                nc.vector.tensor_tensor(av[:], cpv[:, :, :, :, 0],
                                        cpv[:, :, :, :, 1], op=MAXO)
                nc.vector.tensor_tensor(pg[:], av[:, 0], av[:, 1], op=MAXO)
            else:
                # one DVE 6D TensorReduce does the whole 4:1
                pv = ps2.rearrange("p (pl o s tc) -> p o s pl tc", pl=2,
                                   o=20, s=4)
                nc.vector.reduce_max(pg.rearrange("p (o s) -> p o s", o=20),
                                     pv, axis=AX.XY)
            return pg

        conv2_state = {"pending": None}

        def flush_transpose():
            """Emit the delayed transpose (and evict on the 4th of a half)."""
            if conv2_state["pending"] is None:
                return
            g, b, pg = conv2_state["pending"]
            conv2_state["pending"] = None
            bh, i = divmod(b, 4)
            if i == 0:
                conv2_state["tp2"] = tpf_pool.tile([80, 512], FP16,
                                                   name="tp2", tag="tpf")
            tp2 = conv2_state["tp2"]
            nc.tensor.transpose(tp2[:, i * 128:(i + 1) * 128], pg[:],
                                identb[:])
            if i == 3:
                dst = f_g[g][:, bh * 512:(bh + 1) * 512]
                nc.scalar.activation(dst, tp2[:], AF.Relu,
                                     bias=b2_sb[:, 0:1])

        def emit_conv2_block(g, b):
            """One conv2 block; its transpose is delayed one block so the
            PE never waits on the DVE->GpSimd pool2 chain."""
            pg = conv2_block(g, b)
            flush_transpose()
            conv2_state["pending"] = (g, b, pg)


        def emit_fc1(cc):
            psf1 = tpf_pool.tile([50, 512], F32, name="psf1", tag="tpf")
            for g in range(4):
                nc.tensor.matmul(psf1[:], wfc1_sb[:, g * 50:(g + 1) * 50],
                                 f_g[g][:, cc * 512:(cc + 1) * 512],
                                 start=(g == 0), stop=(g == 3))
            dst = fc1o[:, cc * 512:(cc + 1) * 512]
            nc.scalar.activation(dst, psf1[:], AF.Relu,
                                 bias=bf1_sb[:, 0:1])

        def emit_fc2_half(hb):
            """fc2 + log-softmax for blocks hb*4 .. hb*4+3, fully fused."""
            psf2 = ps2_pool.tile([128, 40], F32, name="psf2", tag="ps2")
            for k in range(4):
                b = hb * 4 + k
                nc.tensor.matmul(psf2[:, k * 10:(k + 1) * 10],
                                 fc1o[:, b * 128:(b + 1) * 128],
                                 wfc2_sb[:], start=True, stop=True)
            tslice = t1_all[:, hb * 40:hb * 40 + 40]
            nc.vector.tensor_add(tslice, psf2[:], bc2x4_sb[:])
            e_all = sm_pool.tile([128, 40], F32, name="e_all", tag="e_all")
            nc.scalar.activation(e_all[:], tslice, AF.Exp)
            se = sm_pool.tile([128, 4], F32, name="se", tag="se")
            nc.vector.reduce_sum(
                se[:], e_all.rearrange("p (b t) -> p b t", t=10), axis=AX.X)
            ls = sm_pool.tile([128, 4], F32, name="ls", tag="ls")
            nc.scalar.activation(ls[:], se[:], AF.Ln)
            yo = sm_pool.tile([128, 40], F32, name="yo", tag="yo")
            for k in range(4):
                nc.vector.tensor_scalar_sub(
                    yo[:, k * 10:k * 10 + 10],
                    tslice[:, k * 10:k * 10 + 10], ls[:, k:k + 1])
            nc.sync.dma_start(
                y[hb * 512:(hb + 1) * 512]
                .rearrange("(blk p) c -> p blk c", p=128),
                yo.rearrange("p (blk c) -> p blk c", c=10))

        # ---- software pipeline ----
        # Phase A: c=0 waves of w=0..2 (pure conv1, two-deep psum
        # ping-pong). Phase B: c=1 waves of w=0..2 with the first conv2
        # blocks of g=0 (their sample blocks only need the c=0 chunk).
        # Phase C: w=3..5, each wave slot paired with two conv2 blocks of
        # g=w-3 (b4..7) then g=w-2 (b0..3). Tail: g=3 b4..7 + fc + softmax.
        for w in range(3):
            for h in range(2):
                emit_wave(w, h, 0, "A")
        for w in range(3):
            for h in range(2):
                emit_wave(w, h, 1, "B")
                k = 2 * w + h
                if k >= 2:
                    emit_conv2_block(0, k - 2)
        for w in range(3, 6):
            for h in range(2):
                for c in range(2):
                    emit_wave(w, h, c, None)
                    slot = 2 * h + c
                    if slot < 2:
                        emit_conv2_block(w - 3, 4 + slot * 2)
                        emit_conv2_block(w - 3, 5 + slot * 2)
                    else:
                        emit_conv2_block(w - 2, (slot - 2) * 2)
                        emit_conv2_block(w - 2, (slot - 2) * 2 + 1)

        # tail: last conv2 half-group + fc + epilogue, overlapped
        flush_transpose()
        emit_fc1(0)
        emit_conv2_block(3, 4)
        emit_conv2_block(3, 5)
        emit_fc2_half(0)
        emit_conv2_block(3, 6)
        emit_conv2_block(3, 7)
        flush_transpose()
        emit_fc1(1)
        emit_fc2_half(1)

    nc.compile()
    return nc


_PROGRAM_CACHE = {}


def _get_program(b_core):
    if b_core not in _PROGRAM_CACHE:
        _PROGRAM_CACHE[b_core] = _build(b_core)
    return _PROGRAM_CACHE[b_core]


def make_in_maps(x, weights, b_core=B_CORE, n_cores=N_CORES):
    """Shard x over cores; replicate the (rearranged) parameters."""
    f32 = np.float32
    xr = np.asarray(x, dtype=f32).reshape(-1, 28, 28)
    in_maps = []
    for c in range(n_cores):
        xc = xr[c * b_core:(c + 1) * b_core]  # [b_core, 28, 28]
        xwin = np.empty((12, 128, b_core), np.float16)
        for w in range(6):
            for h in range(2):
                win = xc[:, 4 * w:4 * w + 8, 12 * h:12 * h + 16]
                xwin[w * 2 + h] = win.reshape(b_core, 128).T
        m = {"xw": np.ascontiguousarray(xwin)}
        m.update(weights)
        in_maps.append(m)
    return in_maps


def kernel(**inputs):
    x = np.asarray(inputs["x"], dtype=np.float32)
    weights = _prep_weights(
        np.asarray(inputs["mask_w"], np.float32),
        np.asarray(inputs["conv1_w"], np.float32),
        np.asarray(inputs["conv1_b"], np.float32),
        np.asarray(inputs["conv2_w"], np.float32),
        np.asarray(inputs["conv2_b"], np.float32),
        np.asarray(inputs["fc1_w"], np.float32),
        np.asarray(inputs["fc1_b"], np.float32),
        np.asarray(inputs["fc2_w"], np.float32),
        np.asarray(inputs["fc2_b"], np.float32),
    )
    nc = _get_program(B_CORE)
    in_maps = make_in_maps(x, weights)
    res = run_bass_kernel_spmd(nc, in_maps, list(range(N_CORES)))
    out = np.concatenate([res.results[c]["y"] for c in range(N_CORES)], axis=0)
    return np.ascontiguousarray(out.astype(np.float32))


if __name__ == "__main__":
    rng = np.random.default_rng(0)
    ins = {
        "x": rng.standard_normal((B_TOTAL, 1, 28, 28), dtype=np.float32),
        "mask_w": rng.standard_normal((28, 28), dtype=np.float32) * 0.1,
        "conv1_w": rng.standard_normal((10, 1, 5, 5), dtype=np.float32) * 0.2,
        "conv1_b": rng.standard_normal((10,), dtype=np.float32) * 0.1,
        "conv2_w": rng.standard_normal((20, 10, 5, 5), dtype=np.float32) * 0.06,
        "conv2_b": rng.standard_normal((20,), dtype=np.float32) * 0.1,
        "fc1_w": rng.standard_normal((50, 320), dtype=np.float32) * 0.05,
        "fc1_b": rng.standard_normal((50,), dtype=np.float32) * 0.1,
        "fc2_w": rng.standard_normal((10, 50), dtype=np.float32) * 0.14,
        "fc2_b": rng.standard_normal((10,), dtype=np.float32) * 0.1,
    }
    out = kernel(**ins)
    print(out.shape, out.dtype, out[:2])


# revision 8
# speedup vs baseline: 1.4110x; 1.0576x over previous
"""Trainium2 Bass kernel for nn_CNN_Casual (LeNet-ish CNN, B=8192), v2.

Pure data parallel over 8 NeuronCores: 1024 samples per core, parameters
replicated, one SPMD Bass program.

Design notes (cost model: matmul = out_free x 0.42ns x n_acc; DVE 2x for
fp16 ops (4x for tensor_scalar, all-SBUF); Act 0.83ns/elem, no fast modes;
GpSimd 1.39ns/elem + 95ns launch; Ldweights free):

  conv1  : weights-stationary. Stationary = masked Toeplitz block
           [128 = 8x16 input window, 120 = (P rowpair, o ch, Q colpair)],
           moving = xw window data [128, 512 samples]. 12 windows x 4
           (m rowparity, qp colparity) x 2 chunks = 96 matmuls into 4-bank
           PSUM mega tiles [120, (m, qp, 512)] fp32. No transposes needed:
           output is already [features, samples].
  pool1  : s1 = DVE TT max over qp pairs (cross-bank strided views, one op
           covers both m groups), s2 = GpSimd TT max over m pairs (SBUF
           fp16), then one DVE tensor_scalar (4x mode) applies conv1 bias +
           relu. Partition-remapping SBUF->SBUF DMAs scatter the result
           [120 = (P, o, Q), 1024] into per-row conv2 stationary tiles
           x2[r=2w+P] with layout [(h, o, Q), 1024]. A few (w, h) groups are
           instead drained by Act copies + fp16 TTs (engine balance).
  conv2  : data-stationary: stationary = x2[r] sample-slice [120, 128],
           moving = weight block W_ki [120, 160 = (o2, s2, tc)]; output rows
           2g / 2g+1 are two independent 160-wide 5-step accumulation chains
           in one [128, 320] fp32 PSUM tile (no zero blocks).
  pool2  : s1 = DVE TT max over tc pairs (strided PSUM views) -> fp16, s2 =
           GpSimd TT max over pl pairs -> [128, 80 = (o2, s2)]; PE transpose
           into [80, 512] fp16 PSUM; Act evicts with relu + conv2 bias.
  fc1    : stationary wfc1_g [80, 50] x 4 groups accumulate over moving
           f_g [80, 512]; Act relu+bias evict -> fc1o [50, 1024] fp16.
  fc2    : stationary fc1o [50, 128] per block, moving wfc2 [50, 10].
  softmax: per block t1 = psf2 + (fc2_b - 10) (constant shift is exact for
           log_softmax); batched half-core epilogue: one Exp, windowed
           reduce_sum, one Ln, per-block subtract; DMA out.

The whole thing is software-pipelined over conv1 windows w = 0..5 with
conv2 group g = w - 2 interleaved in emission order so PE never stalls on
PSUM reuse. dtypes: fp16 matmul operands / pooled activations (|x| <= ~30,
end-to-end max rel err ~5e-4 vs fp32 reference); PSUM always fp32 except
transpose tiles; pooling stage 1 reads PSUM fp32.
"""

from contextlib import ExitStack

import numpy as np

import concourse.mybir as mybir
import concourse.tile as tile
from concourse import bacc
from concourse.bass_utils import run_bass_kernel_spmd

F32 = mybir.dt.float32
FP16 = mybir.dt.float16
AF = mybir.ActivationFunctionType
AX = mybir.AxisListType

N_CORES = 8
B_TOTAL = 8192
B_CORE = B_TOTAL // N_CORES  # 1024

# engine-balance knobs
# per-wave count of duos drained via the Act-copy path (cycled)
WAVE_ACT_PATTERN = [2, 2, 1]
# conv2 (g, b) pool2 drain via Act copy instead of a DVE TensorReduce
# (off for the tail half-group so the Act queue stays clear for Exp/Ln)
ASSIST_P2 = lambda g, b: b % 2 == 0 and not (g == 3 and b >= 4)


# --------------------------------------------------------------------------
# Host-side weight preparation (tiny tensors; exact rearrangement only)
# --------------------------------------------------------------------------
def _prep_weights(mask_w, conv1_w, conv1_b, conv2_w, conv2_b, fc1_w, fc1_b,
                  fc2_w, fc2_b):
    f32 = np.float32
    sig = (1.0 / (1.0 + np.exp(-mask_w.astype(f32)))).astype(f32)  # [28,28]

    # conv1 Toeplitz, weights-stationary, mask folded in.
    # block t=(w,h), sub-block (m, qp): [128 = (il, jl), 124 = (slot, o, Q)]
    # out (p = 4w + 2P + m, q = 12h + 2Q + qp); partition slot (P ^ h) * 64
    # (engines cannot remap partitions, so the slot choice makes every
    # pool1-finish write land at its source partition offset);
    # K row (il = 2P + m + ki, jl = 2Q + qp + kj)
    w1m = np.zeros((128, 5952), f32)
    for w in range(6):
        for h in range(2):
            t = 2 * w + h
            for m in range(2):
                for qp in range(2):
                    blk = t * 496 + (2 * m + qp) * 124
                    for P in range(2):
                        for o in range(10):
                            for Q in range(6):
                                col = blk + (P ^ h) * 64 + o * 6 + Q
                                for ki in range(5):
                                    il = 2 * P + m + ki
                                    for kj in range(5):
                                        jl = 2 * Q + qp + kj
                                        w1m[il * 16 + jl, col] = (
                                            conv1_w[o, 0, ki, kj]
                                            * sig[4 * w + il, 12 * h + jl])
    w1m = w1m.astype(np.float16)

    # conv2 blocks per (row parity rho, ki): [124 = (slot, c, Q), 160 =
    # (o2, s2, tc)]; x2[r] partition slot s holds pooled col-half
    # hh = s ^ (r % 2); row = (c, j' = 6*hh + Q); col = output
    # (o2, q2 = 2s2 + tc); nonzero where kj = j' - q2 in [0, 5)
    w2m = np.zeros((2, 5, 124, 160), f32)
    for rho in range(2):
        for sl in range(2):
            hh = sl ^ rho
            for c in range(10):
                for Q in range(6):
                    r = sl * 64 + c * 6 + Q
                    jp = 6 * hh + Q
                    for o2 in range(20):
                        for s2 in range(4):
                            for tc in range(2):
                                q2 = 2 * s2 + tc
                                kj = jp - q2
                                if 0 <= kj < 5:
                                    w2m[rho, :, r, o2 * 8 + s2 * 2 + tc] = \
                                        conv2_w[o2, c, :, kj]
    w2m = np.ascontiguousarray(
        w2m.transpose(2, 0, 1, 3).reshape(124, 1600)).astype(np.float16)

    # fc1 weights per pooled-row group g: rows (o2, s2), torch flatten order
    # of the conv2 activations is (o2, g, s2).
    fc1w4 = fc1_w.reshape(50, 20, 4, 4)  # [m, o2, g, s2]
    wfc1 = np.concatenate(
        [np.ascontiguousarray(fc1w4[:, :, g, :].reshape(50, 80).T)
         for g in range(4)],
        axis=1,
    )  # [80, 200]

    # const blob 1 (fp32): bc2 | b1 | b2 | bf1 | bc2x4 -> [128, 53]
    cst = np.zeros((128, 53), f32)
    cst[:, 0:10] = np.tile(fc2_b.astype(f32).reshape(1, 10) - 10.0, (128, 1))
    b1v = np.repeat(conv1_b.astype(f32), 6)
    cst[0:60, 10] = b1v
    cst[64:124, 10] = b1v
    cst[0:80, 11] = np.repeat(conv2_b.astype(f32), 4)
    cst[0:50, 12] = fc1_b.astype(f32)
    cst[:, 13:53] = np.tile(cst[:, 0:10], (1, 4))

    # const blob 2 (fp16): fc2_w.T | wfc1 -> [80, 210]
    wfcb = np.zeros((80, 210), np.float16)
    wfcb[0:50, 0:10] = fc2_w.T.astype(np.float16)
    wfcb[:, 10:210] = wfc1.astype(np.float16)

    idb = np.eye(128).astype(np.float16)
    return dict(w1m=w1m, w2m=w2m, wfcb=wfcb, cst=cst, idb=idb)


# --------------------------------------------------------------------------
# Device program
# --------------------------------------------------------------------------
def _build(b_core):
    assert b_core == 1024
    n_blk = b_core // 128  # 8

    nc = bacc.Bacc("TRN2", target_bir_lowering=False, debug=False,
                   num_devices=N_CORES)

    xw_d = nc.dram_tensor("xw", [12, 128, b_core], FP16,
                          kind="ExternalInput").ap()
    w1m_d = nc.dram_tensor("w1m", [128, 5952], FP16,
                           kind="ExternalInput").ap()
    w2m_d = nc.dram_tensor("w2m", [124, 1600], FP16, kind="ExternalInput").ap()
    wfcb_d = nc.dram_tensor("wfcb", [80, 210], FP16, kind="ExternalInput").ap()
    cst_d = nc.dram_tensor("cst", [128, 53], F32, kind="ExternalInput").ap()
    idb_d = nc.dram_tensor("idb", [128, 128], FP16, kind="ExternalInput").ap()
    y = nc.dram_tensor("y", [b_core, 10], F32, kind="ExternalOutput").ap()

    MAXO = mybir.AluOpType.max
    ADD = mybir.AluOpType.add

    with tile.TileContext(nc) as tc, ExitStack() as ctx:
        consts = ctx.enter_context(tc.tile_pool(name="consts", bufs=1))
        identb = consts.tile([128, 128], FP16)
        w1m_sb = consts.tile([128, 5952], FP16)
        w2m_sb = consts.tile([124, 1600], FP16)
        wfcb_sb = consts.tile([80, 210], FP16)
        cst_sb = consts.tile([128, 53], F32)

        # pre-load the one activation table that serves Copy/Relu/Exp/Ln
        # (id 6 = natural_log_exp_and_others) so the table-placement pass
        # never needs another load
        nc.scalar.add_instruction(mybir.InstLoadActFuncSet(
            name="preload_act_tbl", act_func_set_id=6, ins=[], outs=[]))

        bc2_sb = cst_sb[:, 0:10]
        bc2x4_sb = cst_sb[:, 13:53]
        b1_sb = cst_sb[0:124, 10:11]
        b2_sb = cst_sb[0:80, 11:12]
        bf1_sb = cst_sb[0:50, 12:13]
        wfc2_sb = wfcb_sb[0:50, 0:10]
        wfc1_sb = wfcb_sb[:, 10:210]

        # xw input: one big tile, window t at free offset t*b_core
        xw_all = consts.tile([128, 12 * b_core], FP16, name="xw_all")
        # conv2 stationary tiles, one per pooled row r
        x2 = [consts.tile([124, b_core], FP16, name=f"x2_{r}")
              for r in range(12)]
        # fc1 moving tiles, one per conv2 row-pair group g
        f_g = [consts.tile([80, b_core], FP16, name=f"f{g}")
               for g in range(4)]
        fc1o = consts.tile([50, b_core], FP16, name="fc1o")
        t1_all = consts.tile([128, 10 * n_blk], F32)

        # Bulk DMA all goes through the otherwise-idle SP queue, in few big
        # transfers (each dma_start costs ~630ns on the serial HWDGE): xw
        # windows in 4 staged chunks, w1m in 2, so the first conv1 matmul
        # only waits for chunk one.
        def prefetch_xw(tlo, thi):
            nc.sync.dma_start(
                xw_all[:, tlo * b_core:thi * b_core]
                .rearrange("p (t n) -> p t n", n=b_core),
                xw_d[tlo:thi].rearrange("t p n -> p t n"))

        prefetch_xw(0, 1)
        nc.sync.dma_start(w1m_sb[:, 0:992], w1m_d[:, 0:992])
        prefetch_xw(1, 3)
        nc.sync.dma_start(w1m_sb[:, 992:2976], w1m_d[:, 992:2976])
        nc.sync.dma_start(cst_sb[:], cst_d)
        prefetch_xw(3, 6)
        nc.sync.dma_start(w2m_sb[:], w2m_d)
        nc.sync.dma_start(wfcb_sb[:], wfcb_d)
        nc.sync.dma_start(identb[:], idb_d)
        prefetch_xw(6, 9)
        nc.sync.dma_start(w1m_sb[:, 2976:5952], w1m_d[:, 2976:5952])
        prefetch_xw(9, 12)

        ps1_pool = ctx.enter_context(tc.tile_pool(name="ps1", bufs=2,
                                                  space="PSUM"))
        ps2_pool = ctx.enter_context(tc.tile_pool(name="ps2", bufs=2,
                                                  space="PSUM"))
        tpf_pool = ctx.enter_context(tc.tile_pool(name="tpf", bufs=2,
                                                  space="PSUM"))
        u_pool = ctx.enter_context(tc.tile_pool(name="u", bufs=3))
        v_pool = ctx.enter_context(tc.tile_pool(name="v", bufs=2))
        p2_pool = ctx.enter_context(tc.tile_pool(name="p2", bufs=5))
        sm_pool = ctx.enter_context(tc.tile_pool(name="sm", bufs=3))

        def conv1_wave(w, h, c, slots):
            """4 matmuls for window t=(w,h), chunk c, into 4 psum slots."""
            t = 2 * w + h
            mov = xw_all[:, t * b_core + c * 512:t * b_core + (c + 1) * 512]
            for i, (m, qp) in enumerate(((0, 0), (0, 1), (1, 0), (1, 1))):
                blk = t * 496 + (2 * m + qp) * 124
                nc.tensor.matmul(
                    slots[i], w1m_sb[:, blk:blk + 124], mov,
                    start=True, stop=True)

        wave_seq = [0]

        def emit_wave(w, h, c, startup):
            """conv1 wave into 4 one-bank psum slots, pool1 drain, then
            bias+relu straight into the x2 chunk halves (partition remap).

            During phase A (pure conv1, no conv2 to fill PE stalls)
            alternate waves borrow the idle ps2/tpf banks so the pipeline
            is two waves deep.
            """
            wave_seq[0] += 1
            duo0 = ps1_pool.tile([124, 1024], F32, name="duo0", tag="mega")
            duo1 = ps1_pool.tile([124, 1024], F32, name="duo1", tag="mega")
            duos = [duo0, duo1]
            slots = [duo0[:, 0:512], duo0[:, 512:1024],
                     duo1[:, 0:512], duo1[:, 512:1024]]
            conv1_wave(w, h, c, slots)
            # legal psum drains (at most ONE psum input per instruction):
            # DVE TensorReduce over a duo's qp pair, or Act copy to fp16
            # followed by a GpSimd TT max, per the balance knobs.
            u = u_pool.tile([124, 1024], FP16, name="u", tag="u")
            uv = u.rearrange("p (m n) -> p m n", m=2)
            act_duos = WAVE_ACT_PATTERN[wave_seq[0] % len(WAVE_ACT_PATTERN)]
            cols = slice(c * 512, (c + 1) * 512)
            for m in range(2):
                if m >= 2 - act_duos:
                    cpy = u_pool.tile([124, 1024], FP16, name="cpy",
                                      tag="cpy")
                    cv = cpy.rearrange("p (q n) -> p q n", q=2)
                    nc.scalar.copy(cv[:], duos[m].rearrange(
                        "p (q n) -> p q n", q=2))
                    nc.vector.tensor_tensor(uv[:, m], cv[:, 0], cv[:, 1],
                                            op=MAXO)
                else:
                    dm = duos[m].rearrange("p (q n) -> p n q", q=2)
                    nc.vector.reduce_max(uv[:, m], dm, axis=AX.X)
            vc = v_pool.tile([124, 512], FP16, name="vc", tag="v")
            nc.vector.tensor_tensor(vc[:], uv[:, 0], uv[:, 1], op=MAXO)
            for sl in range(2):
                row = 2 * w + (sl ^ h)
                # slot 0 covers the 60:64 pad rows too: their psum source is
                # exact zeros (zero stationary columns), so x2 pad rows get a
                # finite constant instead of uninitialized SBUF (NaN x 0 = NaN
                # in the conv2 matmul otherwise)
                pr = slice(0, 64) if sl == 0 else slice(64, 124)
                nc.vector.tensor_scalar(x2[row][pr, cols], vc[pr, :],
                                        b1_sb[pr, 0:1], 0.0,
                                        op0=ADD, op1=MAXO)

        def conv2_block(g, b):
            """conv2 + pool2 for row-pair g, sample block b -> [128,80]."""
            ps2 = ps2_pool.tile([128, 320], F32, name="ps2", tag="ps2")
            for half in range(2):  # pl = half: out row 2g + half
                r0 = 2 * g + half
                for ki in range(5):
                    rho = (r0 + ki) % 2
                    blk = (rho * 5 + ki) * 160
                    nc.tensor.matmul(
                        ps2[:, half * 160:(half + 1) * 160],
                        x2[r0 + ki][:, b * 128:(b + 1) * 128],
                        w2m_sb[:, blk:blk + 160],
                        start=(ki == 0), stop=(ki == 4))
            # pool2: 4:1 over (pl, tc); legal single-psum-input drains only
            pg = p2_pool.tile([128, 80], FP16, name="pg", tag="pg")
            if ASSIST_P2(g, b):
                # Act drains to fp16, maxes on GpSimd + DVE
                cp2 = p2_pool.tile([128, 320], FP16, name="cp2", tag="cp2")
                nc.scalar.copy(cp2[:], ps2[:])
                cpv = cp2.rearrange("p (pl o s tc) -> p pl o s tc",
                                    pl=2, o=20, s=4)
                p2a = p2_pool.tile([128, 160], FP16, name="p2a", tag="p2a")
                av = p2a.rearrange("p (pl c) -> p pl c", pl=2)
                nc.vector.tensor_tensor(av[:], cpv[:, :, :, :, 0],
                                        cpv[:, :, :, :, 1], op=MAXO)
                nc.vector.tensor_tensor(pg[:], av[:, 0], av[:, 1], op=MAXO)
            else:
                # one DVE 6D TensorReduce does the whole 4:1
                pv = ps2.rearrange("p (pl o s tc) -> p o s pl tc", pl=2,
                                   o=20, s=4)
                nc.vector.reduce_max(pg.rearrange("p (o s) -> p o s", o=20),
                                     pv, axis=AX.XY)
            return pg

        conv2_state = {"pending": None}

        def flush_transpose():
            """Emit the delayed transpose (and evict on the 4th of a half)."""
            if conv2_state["pending"] is None:
                return
            g, b, pg = conv2_state["pending"]
            conv2_state["pending"] = None
            bh, i = divmod(b, 4)
            if i == 0:
                conv2_state["tp2"] = tpf_pool.tile([80, 512], FP16,
                                                   name="tp2", tag="tpf")
            tp2 = conv2_state["tp2"]
            nc.tensor.transpose(tp2[:, i * 128:(i + 1) * 128], pg[:],
                                identb[:])
            if i == 3:
                dst = f_g[g][:, bh * 512:(bh + 1) * 512]
                nc.scalar.activation(dst, tp2[:], AF.Relu,
                                     bias=b2_sb[:, 0:1])

        def emit_conv2_block(g, b):
            """One conv2 block; its transpose is delayed one block so the
            PE never waits on the DVE->GpSimd pool2 chain."""
            pg = conv2_block(g, b)
            flush_transpose()
            conv2_state["pending"] = (g, b, pg)


        def emit_fc1(cc):
            psf1 = tpf_pool.tile([50, 512], F32, name="psf1", tag="tpf")
            for g in range(4):
                nc.tensor.matmul(psf1[:], wfc1_sb[:, g * 50:(g + 1) * 50],
                                 f_g[g][:, cc * 512:(cc + 1) * 512],
                                 start=(g == 0), stop=(g == 3))
            dst = fc1o[:, cc * 512:(cc + 1) * 512]
            nc.scalar.activation(dst, psf1[:], AF.Relu,
                                 bias=bf1_sb[:, 0:1])

        def emit_fc2_half(hb):
            """fc2 + log-softmax for blocks hb*4 .. hb*4+3, fully fused."""
            psf2 = ps2_pool.tile([128, 40], F32, name="psf2", tag="ps2")
            for k in range(4):
                b = hb * 4 + k
                nc.tensor.matmul(psf2[:, k * 10:(k + 1) * 10],
                                 fc1o[:, b * 128:(b + 1) * 128],
                                 wfc2_sb[:], start=True, stop=True)
            tslice = t1_all[:, hb * 40:hb * 40 + 40]
            nc.vector.tensor_add(tslice, psf2[:], bc2x4_sb[:])
            e_all = sm_pool.tile([128, 40], F32, name="e_all", tag="e_all")
            nc.scalar.activation(e_all[:], tslice, AF.Exp)
            se = sm_pool.tile([128, 4], F32, name="se", tag="se")
            nc.vector.reduce_sum(
                se[:], e_all.rearrange("p (b t) -> p b t", t=10), axis=AX.X)
            ls = sm_pool.tile([128, 4], F32, name="ls", tag="ls")
            nc.scalar.activation(ls[:], se[:], AF.Ln)
            yo = sm_pool.tile([128, 40], F32, name="yo", tag="yo")
            for k in range(4):
                nc.vector.tensor_scalar_sub(
                    yo[:, k * 10:k * 10 + 10],
                    tslice[:, k * 10:k * 10 + 10], ls[:, k:k + 1])
            nc.sync.dma_start(
                y[hb * 512:(hb + 1) * 512]
                .rearrange("(blk p) c -> p blk c", p=128),
                yo.rearrange("p (blk c) -> p blk c", c=10))

        # ---- software pipeline ----
        # Phase A: c=0 waves of w=0..2 (pure conv1, two-deep psum
        # ping-pong). Phase B: c=1 waves of w=0..2 with the first conv2
        # blocks of g=0 (their sample blocks only need the c=0 chunk).
        # Phase C: w=3..5, each wave slot paired with two conv2 blocks of
        # g=w-3 (b4..7) then g=w-2 (b0..3). Tail: g=3 b4..7 + fc + softmax.
        for w in range(3):
            for h in range(2):
                emit_wave(w, h, 0, "A")
        for w in range(3):
            for h in range(2):
                emit_wave(w, h, 1, "B")
                k = 2 * w + h
                if k >= 2:
                    emit_conv2_block(0, k - 2)
        for w in range(3, 6):
            for h in range(2):
                for c in range(2):
                    emit_wave(w, h, c, None)
                    slot = 2 * h + c
                    if slot < 2:
                        emit_conv2_block(w - 3, 4 + slot * 2)
                        emit_conv2_block(w - 3, 5 + slot * 2)
                    else:
                        emit_conv2_block(w - 2, (slot - 2) * 2)
                        emit_conv2_block(w - 2, (slot - 2) * 2 + 1)

        # tail: last conv2 half-group + fc + epilogue, overlapped
        flush_transpose()
        emit_fc1(0)
        emit_conv2_block(3, 4)
        emit_conv2_block(3, 5)
        emit_fc2_half(0)
        emit_conv2_block(3, 6)
        emit_conv2_block(3, 7)
        flush_transpose()
        emit_fc1(1)
        emit_fc2_half(1)

    nc.compile()
    return nc


_PROGRAM_CACHE = {}


def _get_program(b_core):
    if b_core not in _PROGRAM_CACHE:
        _PROGRAM_CACHE[b_core] = _build(b_core)
    return _PROGRAM_CACHE[b_core]


def make_in_maps(x, weights, b_core=B_CORE, n_cores=N_CORES):
    """Shard x over cores; replicate the (rearranged) parameters."""
    f32 = np.float32
    xr = np.asarray(x, dtype=f32).reshape(-1, 28, 28)
    in_maps = []
    for c in range(n_cores):
        xc = xr[c * b_core:(c + 1) * b_core]  # [b_core, 28, 28]
        xwin = np.empty((12, 128, b_core), np.float16)
        for w in range(6):
            for h in range(2):
                win = xc[:, 4 * w:4 * w + 8, 12 * h:12 * h + 16]
                xwin[w * 2 + h] = win.reshape(b_core, 128).T
        m = {"xw": np.ascontiguousarray(xwin)}
        m.update(weights)
        in_maps.append(m)
    return in_maps


def kernel(**inputs):
    x = np.asarray(inputs["x"], dtype=np.float32)
    weights = _prep_weights(
        np.asarray(inputs["mask_w"], np.float32),
        np.asarray(inputs["conv1_w"], np.float32),
        np.asarray(inputs["conv1_b"], np.float32),
        np.asarray(inputs["conv2_w"], np.float32),
        np.asarray(inputs["conv2_b"], np.float32),
        np.asarray(inputs["fc1_w"], np.float32),
        np.asarray(inputs["fc1_b"], np.float32),
        np.asarray(inputs["fc2_w"], np.float32),
        np.asarray(inputs["fc2_b"], np.float32),
    )
    nc = _get_program(B_CORE)
    in_maps = make_in_maps(x, weights)
    res = run_bass_kernel_spmd(nc, in_maps, list(range(N_CORES)))
    out = np.concatenate([res.results[c]["y"] for c in range(N_CORES)], axis=0)
    return np.ascontiguousarray(out.astype(np.float32))


if __name__ == "__main__":
    rng = np.random.default_rng(0)
    ins = {
        "x": rng.standard_normal((B_TOTAL, 1, 28, 28), dtype=np.float32),
        "mask_w": rng.standard_normal((28, 28), dtype=np.float32) * 0.1,
        "conv1_w": rng.standard_normal((10, 1, 5, 5), dtype=np.float32) * 0.2,
        "conv1_b": rng.standard_normal((10,), dtype=np.float32) * 0.1,
        "conv2_w": rng.standard_normal((20, 10, 5, 5), dtype=np.float32) * 0.06,
        "conv2_b": rng.standard_normal((20,), dtype=np.float32) * 0.1,
        "fc1_w": rng.standard_normal((50, 320), dtype=np.float32) * 0.05,
        "fc1_b": rng.standard_normal((50,), dtype=np.float32) * 0.1,
        "fc2_w": rng.standard_normal((10, 50), dtype=np.float32) * 0.14,
        "fc2_b": rng.standard_normal((10,), dtype=np.float32) * 0.1,
    }
    out = kernel(**ins)
    print(out.shape, out.dtype, out[:2])


# revision 9
# speedup vs baseline: 1.4164x; 1.0038x over previous
"""Trainium2 Bass kernel for nn_CNN_Casual (LeNet-ish CNN, B=8192), v2.

Pure data parallel over 8 NeuronCores: 1024 samples per core, parameters
replicated, one SPMD Bass program.

Design notes (cost model: matmul = out_free x 0.42ns x n_acc; DVE 2x for
fp16 ops (4x for tensor_scalar, all-SBUF); Act 0.83ns/elem, no fast modes;
GpSimd 1.39ns/elem + 95ns launch; Ldweights free):

  conv1  : weights-stationary. Stationary = masked Toeplitz block
           [128 = 8x16 input window, 120 = (P rowpair, o ch, Q colpair)],
           moving = xw window data [128, 512 samples]. 12 windows x 4
           (m rowparity, qp colparity) x 2 chunks = 96 matmuls into 4-bank
           PSUM mega tiles [120, (m, qp, 512)] fp32. No transposes needed:
           output is already [features, samples].
  pool1  : s1 = DVE TT max over qp pairs (cross-bank strided views, one op
           covers both m groups), s2 = GpSimd TT max over m pairs (SBUF
           fp16), then one DVE tensor_scalar (4x mode) applies conv1 bias +
           relu. Partition-remapping SBUF->SBUF DMAs scatter the result
           [120 = (P, o, Q), 1024] into per-row conv2 stationary tiles
           x2[r=2w+P] with layout [(h, o, Q), 1024]. A few (w, h) groups are
           instead drained by Act copies + fp16 TTs (engine balance).
  conv2  : data-stationary: stationary = x2[r] sample-slice [120, 128],
           moving = weight block W_ki [120, 160 = (o2, s2, tc)]; output rows
           2g / 2g+1 are two independent 160-wide 5-step accumulation chains
           in one [128, 320] fp32 PSUM tile (no zero blocks).
  pool2  : s1 = DVE TT max over tc pairs (strided PSUM views) -> fp16, s2 =
           GpSimd TT max over pl pairs -> [128, 80 = (o2, s2)]; PE transpose
           into [80, 512] fp16 PSUM; Act evicts with relu + conv2 bias.
  fc1    : stationary wfc1_g [80, 50] x 4 groups accumulate over moving
           f_g [80, 512]; Act relu+bias evict -> fc1o [50, 1024] fp16.
  fc2    : stationary fc1o [50, 128] per block, moving wfc2 [50, 10].
  softmax: per block t1 = psf2 + (fc2_b - 10) (constant shift is exact for
           log_softmax); batched half-core epilogue: one Exp, windowed
           reduce_sum, one Ln, per-block subtract; DMA out.

The whole thing is software-pipelined over conv1 windows w = 0..5 with
conv2 group g = w - 2 interleaved in emission order so PE never stalls on
PSUM reuse. dtypes: fp16 matmul operands / pooled activations (|x| <= ~30,
end-to-end max rel err ~5e-4 vs fp32 reference); PSUM always fp32 except
transpose tiles; pooling stage 1 reads PSUM fp32.
"""

from contextlib import ExitStack

import numpy as np

import concourse.mybir as mybir
import concourse.tile as tile
from concourse import bacc
from concourse.bass_utils import run_bass_kernel_spmd

F32 = mybir.dt.float32
FP16 = mybir.dt.float16
AF = mybir.ActivationFunctionType
AX = mybir.AxisListType

N_CORES = 8
B_TOTAL = 8192
B_CORE = B_TOTAL // N_CORES  # 1024

# engine-balance knobs
# per-wave count of duos drained via the Act-copy path (cycled)
WAVE_ACT_PATTERN = [2, 2, 1]
# conv2 (g, b) pool2 drain via Act copy instead of a DVE TensorReduce
# (off for the tail half-group so the Act queue stays clear for Exp/Ln)
ASSIST_P2 = lambda g, b: b % 2 == 0 and not (g == 3 and b >= 4)


# --------------------------------------------------------------------------
# Host-side weight preparation (tiny tensors; exact rearrangement only)
# --------------------------------------------------------------------------
def _prep_weights(mask_w, conv1_w, conv1_b, conv2_w, conv2_b, fc1_w, fc1_b,
                  fc2_w, fc2_b):
    f32 = np.float32
    sig = (1.0 / (1.0 + np.exp(-mask_w.astype(f32)))).astype(f32)  # [28,28]

    # conv1 Toeplitz, weights-stationary, mask folded in.
    # block t=(w,h), sub-block (m, qp): [128 = (il, jl), 124 = (slot, o, Q)]
    # out (p = 4w + 2P + m, q = 12h + 2Q + qp); partition slot (P ^ h) * 64
    # (engines cannot remap partitions, so the slot choice makes every
    # pool1-finish write land at its source partition offset);
    # K row (il = 2P + m + ki, jl = 2Q + qp + kj)
    w1m = np.zeros((128, 5952), f32)
    for w in range(6):
        for h in range(2):
            t = 2 * w + h
            for m in range(2):
                for qp in range(2):
                    blk = t * 496 + (2 * m + qp) * 124
                    for P in range(2):
                        for o in range(10):
                            for Q in range(6):
                                col = blk + (P ^ h) * 64 + o * 6 + Q
                                for ki in range(5):
                                    il = 2 * P + m + ki
                                    for kj in range(5):
                                        jl = 2 * Q + qp + kj
                                        w1m[il * 16 + jl, col] = (
                                            conv1_w[o, 0, ki, kj]
                                            * sig[4 * w + il, 12 * h + jl])
    w1m = w1m.astype(np.float16)

    # conv2 blocks per (row parity rho, ki): [124 = (slot, c, Q), 160 =
    # (o2, s2, tc)]; x2[r] partition slot s holds pooled col-half
    # hh = s ^ (r % 2); row = (c, j' = 6*hh + Q); col = output
    # (o2, q2 = 2s2 + tc); nonzero where kj = j' - q2 in [0, 5)
    w2m = np.zeros((2, 5, 124, 160), f32)
    for rho in range(2):
        for sl in range(2):
            hh = sl ^ rho
            for c in range(10):
                for Q in range(6):
                    r = sl * 64 + c * 6 + Q
                    jp = 6 * hh + Q
                    for o2 in range(20):
                        for s2 in range(4):
                            for tc in range(2):
                                q2 = 2 * s2 + tc
                                kj = jp - q2
                                if 0 <= kj < 5:
                                    w2m[rho, :, r, o2 * 8 + s2 * 2 + tc] = \
                                        conv2_w[o2, c, :, kj]
    w2m = np.ascontiguousarray(
        w2m.transpose(2, 0, 1, 3).reshape(124, 1600)).astype(np.float16)

    # fc1 weights per pooled-row group g: rows (o2, s2), torch flatten order
    # of the conv2 activations is (o2, g, s2).
    fc1w4 = fc1_w.reshape(50, 20, 4, 4)  # [m, o2, g, s2]
    wfc1 = np.concatenate(
        [np.ascontiguousarray(fc1w4[:, :, g, :].reshape(50, 80).T)
         for g in range(4)],
        axis=1,
    )  # [80, 200]

    # const blob 1 (fp32): bc2 | b1 | b2 | bf1 | bc2x4 -> [128, 53]
    cst = np.zeros((128, 53), f32)
    cst[:, 0:10] = np.tile(fc2_b.astype(f32).reshape(1, 10) - 10.0, (128, 1))
    b1v = np.repeat(conv1_b.astype(f32), 6)
    cst[0:60, 10] = b1v
    cst[64:124, 10] = b1v
    cst[0:80, 11] = np.repeat(conv2_b.astype(f32), 4)
    cst[0:50, 12] = fc1_b.astype(f32)
    cst[:, 13:53] = np.tile(cst[:, 0:10], (1, 4))

    # const blob 2 (fp16): fc2_w.T | wfc1 -> [80, 210]
    wfcb = np.zeros((80, 210), np.float16)
    wfcb[0:50, 0:10] = fc2_w.T.astype(np.float16)
    wfcb[:, 10:210] = wfc1.astype(np.float16)

    idb = np.eye(128).astype(np.float16)
    return dict(w1m=w1m, w2m=w2m, wfcb=wfcb, cst=cst, idb=idb)


# --------------------------------------------------------------------------
# Device program
# --------------------------------------------------------------------------
def _build(b_core):
    assert b_core == 1024
    n_blk = b_core // 128  # 8

    nc = bacc.Bacc("TRN2", target_bir_lowering=False, debug=False,
                   num_devices=N_CORES)

    xw_d = nc.dram_tensor("xw", [12, 128, b_core], FP16,
                          kind="ExternalInput").ap()
    w1m_d = nc.dram_tensor("w1m", [128, 5952], FP16,
                           kind="ExternalInput").ap()
    w2m_d = nc.dram_tensor("w2m", [124, 1600], FP16, kind="ExternalInput").ap()
    wfcb_d = nc.dram_tensor("wfcb", [80, 210], FP16, kind="ExternalInput").ap()
    cst_d = nc.dram_tensor("cst", [128, 53], F32, kind="ExternalInput").ap()
    idb_d = nc.dram_tensor("idb", [128, 128], FP16, kind="ExternalInput").ap()
    y = nc.dram_tensor("y", [b_core, 10], F32, kind="ExternalOutput").ap()

    MAXO = mybir.AluOpType.max
    ADD = mybir.AluOpType.add

    with tile.TileContext(nc) as tc, ExitStack() as ctx:
        consts = ctx.enter_context(tc.tile_pool(name="consts", bufs=1))
        identb = consts.tile([128, 128], FP16)
        w1m_sb = consts.tile([128, 5952], FP16)
        w2m_sb = consts.tile([124, 1600], FP16)
        wfcb_sb = consts.tile([80, 210], FP16)
        cst_sb = consts.tile([128, 53], F32)

        # pre-load the one activation table that serves Copy/Relu/Exp/Ln
        # (id 6 = natural_log_exp_and_others) so the table-placement pass
        # never needs another load
        nc.scalar.add_instruction(mybir.InstLoadActFuncSet(
            name="preload_act_tbl", act_func_set_id=6, ins=[], outs=[]))

        bc2_sb = cst_sb[:, 0:10]
        bc2x4_sb = cst_sb[:, 13:53]
        b1_sb = cst_sb[0:124, 10:11]
        b2_sb = cst_sb[0:80, 11:12]
        bf1_sb = cst_sb[0:50, 12:13]
        wfc2_sb = wfcb_sb[0:50, 0:10]
        wfc1_sb = wfcb_sb[:, 10:210]

        # xw input: one big tile, window t at free offset t*b_core
        xw_all = consts.tile([128, 12 * b_core], FP16, name="xw_all")
        # conv2 stationary tiles, one per pooled row r
        x2 = [consts.tile([124, b_core], FP16, name=f"x2_{r}")
              for r in range(12)]
        # fc1 moving tiles, one per conv2 row-pair group g
        f_g = [consts.tile([80, b_core], FP16, name=f"f{g}")
               for g in range(4)]
        fc1o = consts.tile([50, b_core], FP16, name="fc1o")
        t1_all = consts.tile([128, 10 * n_blk], F32)

        # Bulk DMA all goes through the otherwise-idle SP queue, in few big
        # transfers (each dma_start costs ~630ns on the serial HWDGE): xw
        # windows in 4 staged chunks, w1m in 2, so the first conv1 matmul
        # only waits for chunk one.
        def prefetch_xw(tlo, thi):
            nc.sync.dma_start(
                xw_all[:, tlo * b_core:thi * b_core]
                .rearrange("p (t n) -> p t n", n=b_core),
                xw_d[tlo:thi].rearrange("t p n -> p t n"))

        prefetch_xw(0, 1)
        nc.sync.dma_start(w1m_sb[:, 0:992], w1m_d[:, 0:992])
        prefetch_xw(1, 3)
        nc.sync.dma_start(w1m_sb[:, 992:2976], w1m_d[:, 992:2976])
        nc.sync.dma_start(cst_sb[:], cst_d)
        prefetch_xw(3, 6)
        nc.sync.dma_start(w2m_sb[:], w2m_d)
        nc.sync.dma_start(wfcb_sb[:], wfcb_d)
        nc.sync.dma_start(identb[:], idb_d)
        prefetch_xw(6, 9)
        nc.sync.dma_start(w1m_sb[:, 2976:5952], w1m_d[:, 2976:5952])
        prefetch_xw(9, 12)

        ps1_pool = ctx.enter_context(tc.tile_pool(name="ps1", bufs=2,
                                                  space="PSUM"))
        ps2_pool = ctx.enter_context(tc.tile_pool(name="ps2", bufs=2,
                                                  space="PSUM"))
        tpf_pool = ctx.enter_context(tc.tile_pool(name="tpf", bufs=2,
                                                  space="PSUM"))
        u_pool = ctx.enter_context(tc.tile_pool(name="u", bufs=5))
        v_pool = ctx.enter_context(tc.tile_pool(name="v", bufs=3))
        p2_pool = ctx.enter_context(tc.tile_pool(name="p2", bufs=7))
        sm_pool = ctx.enter_context(tc.tile_pool(name="sm", bufs=3))

        def conv1_wave(w, h, c, slots):
            """4 matmuls for window t=(w,h), chunk c, into 4 psum slots."""
            t = 2 * w + h
            mov = xw_all[:, t * b_core + c * 512:t * b_core + (c + 1) * 512]
            for i, (m, qp) in enumerate(((0, 0), (0, 1), (1, 0), (1, 1))):
                blk = t * 496 + (2 * m + qp) * 124
                nc.tensor.matmul(
                    slots[i], w1m_sb[:, blk:blk + 124], mov,
                    start=True, stop=True)

        wave_seq = [0]

        def emit_wave(w, h, c, startup):
            """conv1 wave into 4 one-bank psum slots, pool1 drain, then
            bias+relu straight into the x2 chunk halves (partition remap).

            During phase A (pure conv1, no conv2 to fill PE stalls)
            alternate waves borrow the idle ps2/tpf banks so the pipeline
            is two waves deep.
            """
            wave_seq[0] += 1
            duo0 = ps1_pool.tile([124, 1024], F32, name="duo0", tag="mega")
            duo1 = ps1_pool.tile([124, 1024], F32, name="duo1", tag="mega")
            duos = [duo0, duo1]
            slots = [duo0[:, 0:512], duo0[:, 512:1024],
                     duo1[:, 0:512], duo1[:, 512:1024]]
            conv1_wave(w, h, c, slots)
            # legal psum drains (at most ONE psum input per instruction):
            # DVE TensorReduce over a duo's qp pair, or Act copy to fp16
            # followed by a GpSimd TT max, per the balance knobs.
            u = u_pool.tile([124, 1024], FP16, name="u", tag="u")
            uv = u.rearrange("p (m n) -> p m n", m=2)
            act_duos = WAVE_ACT_PATTERN[wave_seq[0] % len(WAVE_ACT_PATTERN)]
            cols = slice(c * 512, (c + 1) * 512)
            for m in range(2):
                if m >= 2 - act_duos:
                    cpy = u_pool.tile([124, 1024], FP16, name="cpy",
                                      tag="cpy")
                    cv = cpy.rearrange("p (q n) -> p q n", q=2)
                    nc.scalar.copy(cv[:], duos[m].rearrange(
                        "p (q n) -> p q n", q=2))
                    nc.vector.tensor_tensor(uv[:, m], cv[:, 0], cv[:, 1],
                                            op=MAXO)
                else:
                    dm = duos[m].rearrange("p (q n) -> p n q", q=2)
                    nc.vector.reduce_max(uv[:, m], dm, axis=AX.X)
            vc = v_pool.tile([124, 512], FP16, name="vc", tag="v")
            nc.vector.tensor_tensor(vc[:], uv[:, 0], uv[:, 1], op=MAXO)
            for sl in range(2):
                row = 2 * w + (sl ^ h)
                # slot 0 covers the 60:64 pad rows too: their psum source is
                # exact zeros (zero stationary columns), so x2 pad rows get a
                # finite constant instead of uninitialized SBUF (NaN x 0 = NaN
                # in the conv2 matmul otherwise)
                pr = slice(0, 64) if sl == 0 else slice(64, 124)
                nc.vector.tensor_scalar(x2[row][pr, cols], vc[pr, :],
                                        b1_sb[pr, 0:1], 0.0,
                                        op0=ADD, op1=MAXO)

        def conv2_block(g, b):
            """conv2 + pool2 for row-pair g, sample block b -> [128,80]."""
            ps2 = ps2_pool.tile([128, 320], F32, name="ps2", tag="ps2")
            for half in range(2):  # pl = half: out row 2g + half
                r0 = 2 * g + half
                for ki in range(5):
                    rho = (r0 + ki) % 2
                    blk = (rho * 5 + ki) * 160
                    nc.tensor.matmul(
                        ps2[:, half * 160:(half + 1) * 160],
                        x2[r0 + ki][:, b * 128:(b + 1) * 128],
                        w2m_sb[:, blk:blk + 160],
                        start=(ki == 0), stop=(ki == 4))
            # pool2: 4:1 over (pl, tc); legal single-psum-input drains only
            pg = p2_pool.tile([128, 80], FP16, name="pg", tag="pg")
            if ASSIST_P2(g, b):
                # Act drains to fp16, maxes on GpSimd + DVE
                cp2 = p2_pool.tile([128, 320], FP16, name="cp2", tag="cp2")
                nc.scalar.copy(cp2[:], ps2[:])
                cpv = cp2.rearrange("p (pl o s tc) -> p pl o s tc",
                                    pl=2, o=20, s=4)
                p2a = p2_pool.tile([128, 160], FP16, name="p2a", tag="p2a")
                av = p2a.rearrange("p (pl c) -> p pl c", pl=2)
                nc.vector.tensor_tensor(av[:], cpv[:, :, :, :, 0],
                                        cpv[:, :, :, :, 1], op=MAXO)
                nc.vector.tensor_tensor(pg[:], av[:, 0], av[:, 1], op=MAXO)
            else:
                # one DVE 6D TensorReduce does the whole 4:1
                pv = ps2.rearrange("p (pl o s tc) -> p o s pl tc", pl=2,
                                   o=20, s=4)
                nc.vector.reduce_max(pg.rearrange("p (o s) -> p o s", o=20),
                                     pv, axis=AX.XY)
            return pg

        conv2_state = {"pending": None}

        def flush_transpose():
            """Emit the delayed transpose (and evict on the 4th of a half)."""
            if conv2_state["pending"] is None:
                return
            g, b, pg = conv2_state["pending"]
            conv2_state["pending"] = None
            bh, i = divmod(b, 4)
            if i == 0:
                conv2_state["tp2"] = tpf_pool.tile([80, 512], FP16,
                                                   name="tp2", tag="tpf")
            tp2 = conv2_state["tp2"]
            nc.tensor.transpose(tp2[:, i * 128:(i + 1) * 128], pg[:],
                                identb[:])
            if i == 3:
                dst = f_g[g][:, bh * 512:(bh + 1) * 512]
                if g == 3 and bh == 1:
                    nc.vector.tensor_scalar(dst, tp2[:], b2_sb[:, 0:1],
                                            0.0, op0=ADD, op1=MAXO)
                else:
                    nc.scalar.activation(dst, tp2[:], AF.Relu,
                                         bias=b2_sb[:, 0:1])

        def emit_conv2_block(g, b):
            """One conv2 block; its transpose is delayed one block so the
            PE never waits on the DVE->GpSimd pool2 chain."""
            pg = conv2_block(g, b)
            flush_transpose()
            conv2_state["pending"] = (g, b, pg)


        def emit_fc1(cc):
            psf1 = tpf_pool.tile([50, 512], F32, name="psf1", tag="tpf")
            for g in range(4):
                nc.tensor.matmul(psf1[:], wfc1_sb[:, g * 50:(g + 1) * 50],
                                 f_g[g][:, cc * 512:(cc + 1) * 512],
                                 start=(g == 0), stop=(g == 3))
            dst = fc1o[:, cc * 512:(cc + 1) * 512]
            if cc == 1:
                nc.vector.tensor_scalar(dst, psf1[:], bf1_sb[:, 0:1], 0.0,
                                        op0=ADD, op1=MAXO)
            else:
                nc.scalar.activation(dst, psf1[:], AF.Relu,
                                     bias=bf1_sb[:, 0:1])

        def emit_fc2_half(hb):
            """fc2 + log-softmax for blocks hb*4 .. hb*4+3, fully fused."""
            psf2 = ps2_pool.tile([128, 40], F32, name="psf2", tag="ps2")
            for k in range(4):
                b = hb * 4 + k
                nc.tensor.matmul(psf2[:, k * 10:(k + 1) * 10],
                                 fc1o[:, b * 128:(b + 1) * 128],
                                 wfc2_sb[:], start=True, stop=True)
            tslice = t1_all[:, hb * 40:hb * 40 + 40]
            nc.vector.tensor_add(tslice, psf2[:], bc2x4_sb[:])
            e_all = sm_pool.tile([128, 40], F32, name="e_all", tag="e_all")
            nc.scalar.activation(e_all[:], tslice, AF.Exp)
            se = sm_pool.tile([128, 4], F32, name="se", tag="se")
            nc.vector.reduce_sum(
                se[:], e_all.rearrange("p (b t) -> p b t", t=10), axis=AX.X)
            ls = sm_pool.tile([128, 4], F32, name="ls", tag="ls")
            nc.scalar.activation(ls[:], se[:], AF.Ln)
            yo = sm_pool.tile([128, 40], F32, name="yo", tag="yo")
            for k in range(4):
                nc.vector.tensor_scalar_sub(
                    yo[:, k * 10:k * 10 + 10],
                    tslice[:, k * 10:k * 10 + 10], ls[:, k:k + 1])
            nc.sync.dma_start(
                y[hb * 512:(hb + 1) * 512]
                .rearrange("(blk p) c -> p blk c", p=128),
                yo.rearrange("p (blk c) -> p blk c", c=10))

        # ---- software pipeline ----
        # Phase A: c=0 waves of w=0..2 (pure conv1, two-deep psum
        # ping-pong). Phase B: c=1 waves of w=0..2 with the first conv2
        # blocks of g=0 (their sample blocks only need the c=0 chunk).
        # Phase C: w=3..5, each wave slot paired with two conv2 blocks of
        # g=w-3 (b4..7) then g=w-2 (b0..3). Tail: g=3 b4..7 + fc + softmax.
        for w in range(3):
            for h in range(2):
                emit_wave(w, h, 0, "A")
        for w in range(3):
            for h in range(2):
                emit_wave(w, h, 1, "B")
                k = 2 * w + h
                if k >= 2:
                    emit_conv2_block(0, k - 2)
        for w in range(3, 6):
            for h in range(2):
                for c in range(2):
                    emit_wave(w, h, c, None)
                    slot = 2 * h + c
                    if slot < 2:
                        emit_conv2_block(w - 3, 4 + slot * 2)
                        emit_conv2_block(w - 3, 5 + slot * 2)
                    else:
                        emit_conv2_block(w - 2, (slot - 2) * 2)
                        emit_conv2_block(w - 2, (slot - 2) * 2 + 1)

        # tail: last conv2 half-group + fc + epilogue, overlapped
        flush_transpose()
        emit_fc1(0)
        emit_conv2_block(3, 4)
        emit_conv2_block(3, 5)
        emit_fc2_half(0)
        emit_conv2_block(3, 6)
        emit_conv2_block(3, 7)
        flush_transpose()
        emit_fc1(1)
        emit_fc2_half(1)

    nc.compile()
    return nc


_PROGRAM_CACHE = {}


def _get_program(b_core):
    if b_core not in _PROGRAM_CACHE:
        _PROGRAM_CACHE[b_core] = _build(b_core)
    return _PROGRAM_CACHE[b_core]


def make_in_maps(x, weights, b_core=B_CORE, n_cores=N_CORES):
    """Shard x over cores; replicate the (rearranged) parameters."""
    f32 = np.float32
    xr = np.asarray(x, dtype=f32).reshape(-1, 28, 28)
    in_maps = []
    for c in range(n_cores):
        xc = xr[c * b_core:(c + 1) * b_core]  # [b_core, 28, 28]
        xwin = np.empty((12, 128, b_core), np.float16)
        for w in range(6):
            for h in range(2):
                win = xc[:, 4 * w:4 * w + 8, 12 * h:12 * h + 16]
                xwin[w * 2 + h] = win.reshape(b_core, 128).T
        m = {"xw": np.ascontiguousarray(xwin)}
        m.update(weights)
        in_maps.append(m)
    return in_maps


def kernel(**inputs):
    x = np.asarray(inputs["x"], dtype=np.float32)
    weights = _prep_weights(
        np.asarray(inputs["mask_w"], np.float32),
        np.asarray(inputs["conv1_w"], np.float32),
        np.asarray(inputs["conv1_b"], np.float32),
        np.asarray(inputs["conv2_w"], np.float32),
        np.asarray(inputs["conv2_b"], np.float32),
        np.asarray(inputs["fc1_w"], np.float32),
        np.asarray(inputs["fc1_b"], np.float32),
        np.asarray(inputs["fc2_w"], np.float32),
        np.asarray(inputs["fc2_b"], np.float32),
    )
    nc = _get_program(B_CORE)
    in_maps = make_in_maps(x, weights)
    res = run_bass_kernel_spmd(nc, in_maps, list(range(N_CORES)))
    out = np.concatenate([res.results[c]["y"] for c in range(N_CORES)], axis=0)
    return np.ascontiguousarray(out.astype(np.float32))


if __name__ == "__main__":
    rng = np.random.default_rng(0)
    ins = {
        "x": rng.standard_normal((B_TOTAL, 1, 28, 28), dtype=np.float32),
        "mask_w": rng.standard_normal((28, 28), dtype=np.float32) * 0.1,
        "conv1_w": rng.standard_normal((10, 1, 5, 5), dtype=np.float32) * 0.2,
        "conv1_b": rng.standard_normal((10,), dtype=np.float32) * 0.1,
        "conv2_w": rng.standard_normal((20, 10, 5, 5), dtype=np.float32) * 0.06,
        "conv2_b": rng.standard_normal((20,), dtype=np.float32) * 0.1,
        "fc1_w": rng.standard_normal((50, 320), dtype=np.float32) * 0.05,
        "fc1_b": rng.standard_normal((50,), dtype=np.float32) * 0.1,
        "fc2_w": rng.standard_normal((10, 50), dtype=np.float32) * 0.14,
        "fc2_b": rng.standard_normal((10,), dtype=np.float32) * 0.1,
    }
    out = kernel(**ins)
    print(out.shape, out.dtype, out[:2])
